# revision 23
# baseline (speedup 1.0000x reference)
"""GATClassifier (2x GATConv + mean-pool + linear) on 8 Trainium2 NeuronCores.

v4: unified-table design.

- No layer-1 AllGather: every core redundantly builds the FULL layer-1 table
  (x@W1 for all 50176 padded nodes is ~100us spread over PE/Act/DVE, far
  cheaper than a 38MB collective).
- ONE layer-2 AllGather of the whole shard table (the collective cost model
  and real fabric both reward one large transfer over two mid-size ones).
  Collectives ride the Pool queue (the only HW-legal bass engine).
- Tables are plain node-major [50176 rows x 384 bf16 cols]; gather tokens are
  split at row 31360 (= 5 shards, int16-safe: lo tokens < 31360, hi tokens
  < 18816) with STATIC in_ap bases, so layer 1 and layer 2 share one token
  stream and one one-hot stream.
- Gathers are merged across 2-window groups (slot layout [w0lo|w1lo|w0hi|
  w1hi], each window-half padded to full 128-slot blocks with junk tokens),
  halving the fixed SWDGE cost per gather.
- a_dst routing: no per-edge 256B gather.  A per-layer "panel" gather pulls
  this core's own 6272 eval blocks (two dynamic-count gathers, one of which
  is a no-op, because the shard lives entirely below or above the 31360
  split); a transposed one-hot (StT, host data) routes a_dst[dst] to edge
  slots with one tiny PE matmul per block.
- h columns are c-major (col = c*H + h) so the per-edge message multiply and
  the softmax divide hit the DVE 2x perf mode (packed 4-wide last dim).
- elu's "-1" is folded into the next layer's bias host-side, and elu is
  computed with two Act Relu/Exp ops + one DVE add.

Everything is SPMD-uniform: all core-specific info arrives as data.
"""

import math
import os

import numpy as np

# ---------------------------------------------------------------- constants
N = 50000       # nodes
E = 800000      # directed edges before self loops
IN = 128        # in channels
H = 4           # heads
C = 64          # channels per head
HC = H * C      # 256
G = 64          # graphs
NC_ = 8         # cores
P = 128
SH = 6272       # padded, 128-aligned nodes per shard (49 windows)
NW = SH // P            # 49 windows per core
NP = NC_ * SH           # 50176 padded nodes (global row == node id)
LSPL = 5 * SH           # 31360 lo/hi token split (shard-aligned, < 32768)
NHI = NP - LSPL         # 18816 hi rows
RW = 384                # bf16 cols per table row (768 B)
XC = HC + 2 * H         # 264 meaningful f32 cols of [h|a_src|a_dst]
EC = HC + 4 * H         # 272 bf16 cols written per row
GW = 2                  # windows per gather group


def _bf16():
    import ml_dtypes
    return ml_dtypes.bfloat16


def _wrap16(tok: np.ndarray) -> np.ndarray:
    """dma_gather index layout: token i lives at [i%16, i//16], replicated
    into all 8 groups of 16 partitions."""
    assert tok.size % 16 == 0
    w = tok.reshape(-1, 16).T.astype(np.int16)  # [16, L/16]
    return np.tile(w, (8, 1))                   # [128, L/16]


def _groups():
    gs, w = [], 0
    while w < NW:
        gs.append(list(range(w, min(w + GW, NW))))
        w += GW
    return gs


def _preprocess(edge_index: np.ndarray, batch: np.ndarray):
    """Host-side integer-only preprocessing: shard edges by dst window, sort
    into (window, lo/hi, src) order, pack into per-group gather streams with
    full-block padding, and emit per-core index/dstloc/one-hot-T arrays."""
    src = np.concatenate([edge_index[0], np.arange(N, dtype=np.int64)])
    dst = np.concatenate([edge_index[1], np.arange(N, dtype=np.int64)])
    half = (src >= LSPL).astype(np.int64)
    tok = np.where(half == 0, src, src - LSPL)

    owner = dst // SH
    dl = dst % SH
    wd, dloc = dl // P, dl % P

    counts = np.zeros((NC_, NW, 2), dtype=np.int64)
    per_core = []
    for c in range(NC_):
        m = owner == c
        t_, h_, w_, d_ = tok[m], half[m], wd[m], dloc[m]
        order = np.lexsort((t_, h_, w_))
        t_, h_, w_, d_ = t_[order], h_[order], w_[order], d_[order]
        np.add.at(counts[c], (w_, h_), 1)
        per_core.append((t_, h_, w_, d_))

    maxcnt = counts.max(axis=0)                       # [NW, 2]
    blo = (maxcnt[:, 0] + P - 1) // P
    bhi = (maxcnt[:, 1] + P - 1) // P
    bw = blo + bhi
    totb = int(bw.sum())

    groups = _groups()
    ng = len(groups)
    # per-group block bases; slot layout per group: [w0lo | w1lo | w0hi | w1hi]
    gbase = np.zeros(ng, dtype=np.int64)
    acc = 0
    for gi, ws in enumerate(groups):
        gbase[gi] = acc
        acc += int(bw[ws].sum())
    assert acc == totb
    bwg = np.array([int(bw[ws].sum()) for ws in groups])
    blog = np.array([int(blo[ws].sum()) for ws in groups])
    bwgmax = int(bwg.max())

    # per-window block ranges (global block ids): (lo_range, hi_range)
    wblocks = [None] * NW
    for gi, ws in enumerate(groups):
        b0 = int(gbase[gi])
        lo0 = b0
        hi0 = b0 + int(blo[ws].sum())
        for wi in ws:
            lor = (lo0, lo0 + int(blo[wi]))
            lo0 = lor[1]
            hir = (hi0, hi0 + int(bhi[wi]))
            hi0 = hir[1]
            wblocks[wi] = (lor, hir)

    dstloc = np.full((NC_, P, totb), -1.0, dtype=np.float32)
    ilo_l, ihi_l = [], []
    for c in range(NC_):
        t_, h_, w_, d_ = per_core[c]
        lo_parts, hi_parts = [], []
        for gi, ws in enumerate(groups):
            for hf, bcnt_arr, parts in ((0, blo, lo_parts),
                                        (1, bhi, hi_parts)):
                for wi in ws:
                    m = (w_ == wi) & (h_ == hf)
                    nreal = int(m.sum())
                    nt = int(bcnt_arr[wi]) * P
                    tt = np.zeros(nt, dtype=np.int64)
                    tt[:nreal] = t_[m]
                    dd = np.full(nt, -1.0, dtype=np.float32)
                    dd[:nreal] = d_[m]
                    parts.append(tt)
                    (lor, hir) = wblocks[wi]
                    b0 = lor[0] if hf == 0 else hir[0]
                    bcnt = int(bcnt_arr[wi])
                    if bcnt:
                        dstloc[c, :, b0:b0 + bcnt] = dd.reshape(bcnt, P).T
        ilo_l.append(np.concatenate([_wrap16(x) for x in lo_parts], axis=1))
        ihi_l.append(np.concatenate([_wrap16(x) for x in hi_parts], axis=1))
    ilo = np.stack(ilo_l)
    ihi = np.stack(ihi_l)

    # per-group offsets (in 8-col = 16-token units) into ilo/ihi
    ntlo_g = np.array([int(blo[ws].sum()) * P for ws in groups])
    nthi_g = np.array([int(bhi[ws].sum()) * P for ws in groups])
    lo_off8 = np.concatenate([[0], np.cumsum(ntlo_g // 16)[:-1]]).astype(
        np.int64)
    hi_off8 = np.concatenate([[0], np.cumsum(nthi_g // 16)[:-1]]).astype(
        np.int64)

    # transposed one-hot StT[j, b*128+s] = (dstloc[s, b] == j), bf16
    bf16 = _bf16()
    stT = np.zeros((NC_, P, totb * P), dtype=bf16)
    for c in range(NC_):
        dl_ = dstloc[c]                       # [s=128, b=totb]
        s_idx, b_idx = np.nonzero(dl_ >= 0.0)
        j_idx = dl_[s_idx, b_idx].astype(np.int64)
        stT[c][j_idx, b_idx * P + s_idx] = 1.0

    # both panel gathers are fully static: the inactive side gathers SH
    # safe junk rows (masked out later); no dynamic counts anywhere
    junk = np.arange(SH, dtype=np.int64)
    ipanL = np.zeros((NC_, P, SH // 16), dtype=np.int16)
    ipanH = np.zeros((NC_, P, SH // 16), dtype=np.int16)
    pmask = np.zeros((NC_, P, 2), dtype=np.float32)
    for c in range(NC_):
        rows = np.arange(c * SH, (c + 1) * SH, dtype=np.int64)
        if c * SH >= LSPL:
            ipanH[c] = _wrap16(rows - LSPL)
            ipanL[c] = _wrap16(junk)
            pmask[c, :, 1] = 1.0
        else:
            ipanL[c] = _wrap16(rows)
            ipanH[c] = _wrap16(junk)
            pmask[c, :, 0] = 1.0

    # batch (graph id) per local node slot; -1 on ghost slots
    batchloc = np.full((NC_, P, NW), -1.0, dtype=np.float32)
    for c in range(NC_):
        lo, hi = c * SH, min((c + 1) * SH, N)
        b = np.full(SH, -1.0, dtype=np.float32)
        if hi > lo:
            b[:hi - lo] = batch[lo:hi].astype(np.float32)
        batchloc[c] = b.reshape(NW, P).T

    return dict(
        blo=blo.astype(int), bhi=bhi.astype(int), bw=bw.astype(int),
        totb=totb, groups=groups, gbase=gbase, bwg=bwg, blog=blog,
        bwgmax=bwgmax, wblocks=wblocks,
        ntlo_g=ntlo_g, nthi_g=nthi_g, lo_off8=lo_off8, hi_off8=hi_off8,
        ilo=ilo, ihi=ihi, stT=stT, ipanL=ipanL, ipanH=ipanH, pmask=pmask,
        dstloc=dstloc, batchloc=batchloc,
    )


# c-major permutation: new col c*H+h holds original col h*C+c
_PERM = np.array([h * C + c for c in range(C) for h in range(H)], np.int64)


def _fold(Wm, a_s, a_d, b, perm_rows: bool):
    """[W(c-major cols) | A_src | A_dst] and matching extended bias."""
    K = Wm.shape[0]
    As = np.einsum("khc,hc->kh", Wm.reshape(K, H, C), a_s)
    Ad = np.einsum("khc,hc->kh", Wm.reshape(K, H, C), a_d)
    WR = np.concatenate([Wm[:, _PERM], As, Ad], axis=1).astype(np.float32)
    if perm_rows:
        WR = WR[_PERM]
    be = np.concatenate(
        [b[_PERM], np.einsum("hc,hc->h", b.reshape(H, C), a_s),
         np.einsum("hc,hc->h", b.reshape(H, C), a_d)]
    ).astype(np.float32)                                           # [264]
    return WR, be


def _build(meta, has_b1: bool):
    import concourse.bacc as bacc
    import concourse.mybir as mybir
    import concourse.tile as tile

    kq = int(os.environ.get("KQ", "1"))        # swdge queues
    kbg = int(os.environ.get("KBG", "2"))      # gather pool bufs

    f32 = mybir.dt.float32
    bf = mybir.dt.bfloat16
    i16 = mybir.dt.int16
    i32 = mybir.dt.int32
    Act = mybir.ActivationFunctionType
    Alu = mybir.AluOpType

    groups = meta["groups"]
    blog, bwg, gbase = meta["blog"], meta["bwg"], meta["gbase"]
    ntlo_g, nthi_g = meta["ntlo_g"], meta["nthi_g"]
    lo_off8, hi_off8 = meta["lo_off8"], meta["hi_off8"]
    wblocks = meta["wblocks"]
    TOTB, BWG = meta["totb"], meta["bwgmax"]
    MBW = int(meta["bw"].max())
    NLO8, NHI8 = int((ntlo_g // 16).sum()), int((nthi_g // 16).sum())

    nc = bacc.Bacc("TRN2", target_bir_lowering=False, debug=False,
                   num_devices=NC_, num_swdge_queues=kq)

    grp = [list(range(NC_))]

    def cc(kind, op, ins, outs):
        nc.gpsimd.collective_compute(
            kind, op, replica_groups=grp, ins=ins, outs=outs)

    # ------------------------------------------------------------- tensors
    xT = nc.dram_tensor("xT", [P, NP], bf, kind="ExternalInput")
    W1R = nc.dram_tensor("W1R", [IN, XC], bf, kind="ExternalInput")
    W2Ra = nc.dram_tensor("W2Ra", [P, XC], f32, kind="ExternalInput")
    W2Rb = nc.dram_tensor("W2Rb", [P, XC], f32, kind="ExternalInput")
    b2e = nc.dram_tensor("b2e", [P, XC], f32, kind="ExternalInput")
    Wlin = nc.dram_tensor("Wlin", [P, 4], f32, kind="ExternalInput")
    blin = nc.dram_tensor("blin", [G, 2], f32, kind="ExternalInput")
    iota128 = nc.dram_tensor("iota128", [P, P], bf, kind="ExternalInput")
    iota64 = nc.dram_tensor("iota64", [P, G], f32, kind="ExternalInput")
    ident = nc.dram_tensor("ident", [P, P], f32, kind="ExternalInput")
    identB = nc.dram_tensor("identB", [P, P], bf, kind="ExternalInput")
    dstloc = nc.dram_tensor("dstloc", [P, TOTB], bf, kind="ExternalInput")
    stT = nc.dram_tensor("stT", [P, TOTB * P], bf, kind="ExternalInput")
    idxlo = nc.dram_tensor("idxlo", [P, NLO8], i16, kind="ExternalInput")
    idxhi = nc.dram_tensor("idxhi", [P, NHI8], i16, kind="ExternalInput")
    idxpanL = nc.dram_tensor("idxpanL", [P, SH // 16], i16,
                             kind="ExternalInput")
    idxpanH = nc.dram_tensor("idxpanH", [P, SH // 16], i16,
                             kind="ExternalInput")
    pmask = nc.dram_tensor("pmask", [P, 2], f32, kind="ExternalInput")
    batchloc = nc.dram_tensor("batchloc", [P, NW], f32, kind="ExternalInput")
    if has_b1:
        b1e = nc.dram_tensor("b1e", [P, XC], f32, kind="ExternalInput")

    logits = nc.dram_tensor("logits", [G, 2], f32, kind="ExternalOutput")
    kdbg = os.environ.get("KDBG", "0") == "1"
    if kdbg:
        dbgel = nc.dram_tensor("dbgel", [SH, HC], f32, kind="ExternalOutput")
        dbgob = nc.dram_tensor("dbgob", [SH, HC], f32, kind="ExternalOutput")

    T1u = nc.dram_tensor("T1u", [NP, RW], bf, kind="Internal")
    T2s = nc.dram_tensor("T2s", [SH, RW], bf, kind="Internal")
    T2u = nc.dram_tensor("T2u", [NP, RW], bf, kind="Internal",
                         addr_space="Shared")
    prd = nc.dram_tensor("prd", [G, HC + 1], f32, kind="Internal")
    prs = nc.dram_tensor("prs", [G, HC + 1], f32, kind="Internal",
                         addr_space="Shared")

    with tile.TileContext(nc) as tc:
        with (
            tc.tile_pool(name="const", bufs=1) as cp,
            tc.tile_pool(name="work", bufs=3) as wp,
            tc.tile_pool(name="xw", bufs=2) as xp,
            tc.tile_pool(name="gat", bufs=kbg) as gp,
            tc.tile_pool(name="sel", bufs=2) as sp,
            tc.tile_pool(name="selt", bufs=2) as stp,
            tc.tile_pool(name="pan", bufs=1) as pnp,
            tc.tile_pool(name="adw", bufs=4) as awp,
            tc.tile_pool(name="msg", bufs=2) as mp,
            tc.tile_pool(name="outp", bufs=2) as op_,
            tc.tile_pool(name="ppre", bufs=2, space="PSUM") as ppre,
            tc.tile_pool(name="ptp", bufs=1, space="PSUM") as ptp,
            tc.tile_pool(name="pnum", bufs=2, space="PSUM") as pnum,
            tc.tile_pool(name="ppool", bufs=1, space="PSUM") as ppl,
            tc.tile_pool(name="pad", bufs=1, space="PSUM") as pap,
        ):
            # ---------------------------------------------------- constants
            def cload(dram, dt):
                tl = cp.tile(list(dram.shape), dt, tag=dram.name)
                nc.sync.dma_start(tl[:], dram[:])
                return tl

            w1r_t = cload(W1R, bf)
            w2a_t = cload(W2Ra, f32)
            w2b_t = cload(W2Rb, f32)
            b2e_t = cload(b2e, f32)
            wl_t = cload(Wlin, f32)
            bl_t = cload(blin, f32)
            io64_t = cload(iota64, f32)
            id_t = cload(ident, f32)
            idB_t = cload(identB, bf)
            ilo_t = cload(idxlo, i16)
            ihi_t = cload(idxhi, i16)
            ipanL_t = cload(idxpanL, i16)
            ipanH_t = cload(idxpanH, i16)
            pm_t = cload(pmask, f32)
            bat_t = cload(batchloc, f32)
            if has_b1:
                b1e_t = cload(b1e, f32)
            io1_t = cp.tile([P, 1, P], bf, tag="io1")
            nc.sync.dma_start(io1_t[:, 0, :], iota128[:])
            dst3_t = cp.tile([P, TOTB, 1], bf, tag="dst3")
            nc.sync.dma_start(
                dst3_t[:].rearrange("p b one -> p (b one)"), dstloc[:])

            # --------------------------------------------- layer-1 table
            # (full, built redundantly on every core, in global row order)
            def phase_a():
                nb = math.ceil(NP // P / 4)
                for bi in range(nb):
                    w0 = bi * 4
                    n = min(4, NP // P - w0)
                    g0 = w0 * P
                    xt = xp.tile([P, 4 * P], bf, tag="xt")
                    nc.sync.dma_start(xt[:, 0:n * P], xT[:, g0:g0 + n * P])
                    h4 = xp.tile([P, 4, EC], bf, tag="h4")
                    for k in range(n):
                        ps = ppre.tile([P, XC], f32, space="PSUM", tag="ppre")
                        nc.tensor.matmul(ps[:], lhsT=xt[:, k * P:(k + 1) * P],
                                         rhs=w1r_t[:], start=True, stop=True)
                        if has_b1:
                            nc.vector.tensor_tensor(
                                h4[:, k, 0:HC], ps[:, 0:HC],
                                b1e_t[:, 0:HC], op=Alu.add)
                            nc.vector.tensor_tensor(
                                h4[:, k, HC:EC].bitcast(f32),
                                ps[:, HC:XC], b1e_t[:, HC:XC], op=Alu.add)
                        else:
                            eng = (nc.scalar.copy, nc.vector.tensor_copy
                                   )[(bi * 4 + k) % 2]
                            eng(h4[:, k, 0:HC], ps[:, 0:HC])
                            nc.vector.tensor_copy(
                                h4[:, k, HC:EC].bitcast(f32), ps[:, HC:XC])
                    nc.sync.dma_start(
                        T1u[g0:g0 + n * P, 0:EC].rearrange(
                            "(k p) e -> p k e", p=P),
                        h4[:, 0:n, :])

            # ------------------------------------------------ panel gather
            # own shard's eval blocks -> a_dst[j] per window, [P, NW, H] bf16
            NWH = 25                      # panel windows per pass
            SHH = NWH * P                 # 3200 tokens per pass

            def panel1(Tu):
                adw = awp.tile([P, NW, H], bf, tag="adw")
                # each pass covers up to NWH windows; the inactive side
                # gathers 16 junk tokens per pass and is masked out
                for t0 in range(0, NW, NWH):
                    nwp = min(NWH, NW - t0)
                    ntp = nwp * P
                    ptL = pnp.tile([P, NWH, P], bf, tag="panL")
                    nc.gpsimd.dma_gather(
                        ptL[:, 0:nwp, :], Tu[0:LSPL, HC:HC + P],
                        ipanL_t[:, t0 * 8:(t0 + nwp) * 8],
                        ntp, ntp, P, elem_step=RW, single_packet=False)
                    ptH = pnp.tile([P, NWH, P], bf, tag="panH")
                    nc.gpsimd.dma_gather(
                        ptH[:, 0:nwp, :], Tu[LSPL:NP, HC:HC + P],
                        ipanH_t[:, t0 * 8:(t0 + nwp) * 8],
                        ntp, ntp, P, elem_step=RW, single_packet=False,
                        queue_num=1 % kq)
                    aw = awp.tile([P, NWH, H], f32, tag="aw32")
                    nc.vector.tensor_scalar(
                        aw[:, 0:nwp, :],
                        ptL[:, 0:nwp, :].bitcast(f32)[:, :, H:2 * H],
                        pm_t[:, 0:1], None, op0=Alu.mult)
                    ah = awp.tile([P, NWH, H], f32, tag="ah32")
                    nc.vector.tensor_scalar(
                        ah[:, 0:nwp, :],
                        ptH[:, 0:nwp, :].bitcast(f32)[:, :, H:2 * H],
                        pm_t[:, 1:2], None, op0=Alu.mult)
                    nc.vector.tensor_tensor(adw[:, t0:t0 + nwp, :],
                                            aw[:, 0:nwp, :],
                                            ah[:, 0:nwp, :], op=Alu.add)
                return adw

            def panel2():
                # T2s is core-local: the eval blocks are a plain DMA away
                adw = awp.tile([P, NW, H], bf, tag="adw")
                for t0 in range(0, NW, NWH):
                    nwp = min(NWH, NW - t0)
                    pt = pnp.tile([P, NWH, P], bf, tag="panL")
                    nc.sync.dma_start(
                        pt[:, 0:nwp, :],
                        T2s[t0 * P:(t0 + nwp) * P, HC:HC + P].rearrange(
                            "(w p) c -> p w c", p=P))
                    nc.vector.tensor_copy(
                        adw[:, t0:t0 + nwp, :],
                        pt[:, 0:nwp, :].bitcast(f32)[:, :, H:2 * H])
                return adw

            # shared window loop -----------------------------------------
            def window_loop(Tu, adw, sink):
                for gi, ws in enumerate(groups):
                    BL, BW = int(blog[gi]), int(bwg[gi])
                    b0 = int(gbase[gi])
                    NTL, NTH = int(ntlo_g[gi]), int(nthi_g[gi])
                    Gt = gp.tile([P, BWG, RW], bf, tag="G")
                    nc.gpsimd.dma_gather(
                        Gt[:, 0:BL, :], Tu[0:LSPL, :],
                        ilo_t[:, int(lo_off8[gi]):
                              int(lo_off8[gi]) + NTL // 16],
                        NTL, NTL, RW, single_packet=False)
                    nc.gpsimd.dma_gather(
                        Gt[:, BL:BW, :], Tu[LSPL:NP, :],
                        ihi_t[:, int(hi_off8[gi]):
                              int(hi_off8[gi]) + NTH // 16],
                        NTH, NTH, RW, single_packet=False,
                        queue_num=1 % kq)

                    # one-hot: S[p, b, j] = (j == dstloc[p, b]); and its
                    # transpose StT (host data) for a_dst routing
                    St = sp.tile([P, BWG, P], bf, tag="S")
                    nc.vector.tensor_tensor(
                        St[:, :BW, :],
                        io1_t[:].to_broadcast([P, BW, P]),
                        dst3_t[:, b0:b0 + BW, :].to_broadcast([P, BW, P]),
                        op=Alu.is_equal)
                    StT = stp.tile([P, BWG, P], bf, tag="ST")
                    nc.sync.dma_start(
                        StT[:, 0:BW, :].rearrange("p b s -> p (b s)"),
                        stT[:, b0 * P:(b0 + BW) * P])

                    # a_dst per edge slot: adp[s, b, h] = sum_j StT[j, b, s]
                    # * adw[j, w(b), h]  (tiny per-block PE matmuls)
                    adp = pap.tile([P, BWG, H], f32, space="PSUM", tag="ad")
                    for wi in ws:
                        for r0, r1 in wblocks[wi]:
                            for b in range(r0, r1):
                                nc.tensor.matmul(
                                    adp[:, b - b0, :],
                                    lhsT=StT[:, b - b0, :],
                                    rhs=adw[:, wi, :],
                                    start=True, stop=True)

                    # e = a_src(gathered) + a_dst(routed);
                    # exp(leaky(e)) as exp(0.6*(e + (2/3)|e|))
                    ev = mp.tile([P, BWG, H], f32, tag="ev")
                    GtF = Gt[:].bitcast(f32)
                    nc.vector.tensor_tensor(ev[:, :BW, :],
                                            GtF[:, :BW, P:P + H],
                                            adp[:, :BW, :], op=Alu.add)
                    av = mp.tile([P, BWG, H], f32, tag="av")
                    nc.scalar.activation(av[:, :BW, :], ev[:, :BW, :],
                                         Act.Abs, scale=2.0 / 3.0)
                    nc.vector.tensor_tensor(av[:, :BW, :], ev[:, :BW, :],
                                            av[:, :BW, :], op=Alu.add)
                    nc.vector.tensor_scalar(av[:, :BW, :], av[:, :BW, :],
                                            60.0, None, op0=Alu.min)
                    ex = mp.tile([P, BWG, H], bf, tag="ex")
                    nc.scalar.activation(ex[:, :BW, :], av[:, :BW, :],
                                         Act.Exp, scale=0.6)

                    for wi in ws:
                        # msg = [h * ex | ex]; c-major h -> packed 4-wide
                        # last dim for the DVE fast mode
                        (lor, hir) = wblocks[wi]
                        blocks = list(range(*lor)) + list(range(*hir))
                        nb_w = len(blocks)
                        msg = mp.tile([P, MBW, HC + H], bf, tag="msg")
                        for j0, (r0, r1) in ((0, lor),
                                             (lor[1] - lor[0], hir)):
                            if r1 == r0:
                                continue
                            nbr = r1 - r0
                            nc.vector.tensor_tensor(
                                msg[:, j0:j0 + nbr, 0:HC].rearrange(
                                    "p b (c h) -> p b c h", c=C),
                                Gt[:, r0 - b0:r1 - b0, 0:HC].rearrange(
                                    "p b (c h) -> p b c h", c=C),
                                ex[:, r0 - b0:r1 - b0, :].rearrange(
                                    "p b h -> p b () h").to_broadcast(
                                        [P, nbr, C, H]),
                                op=Alu.mult)
                            nc.scalar.copy(
                                msg[:, j0:j0 + nbr, HC:HC + H],
                                ex[:, r0 - b0:r1 - b0, :])
                        nmp = pnum.tile([P, HC + H], f32, space="PSUM",
                                        tag="nm")
                        for i, b in enumerate(blocks):
                            nc.tensor.matmul(nmp[:], lhsT=St[:, b - b0, :],
                                             rhs=msg[:, i, 0:HC + H],
                                             start=(i == 0),
                                             stop=(i == nb_w - 1))
                        rd = mp.tile([P, H], f32, tag="rd")
                        nc.vector.tensor_scalar(rd[:], nmp[:, HC:HC + H],
                                                1e-30, None, op0=Alu.max)
                        nc.vector.reciprocal(rd[:], rd[:])
                        ob = op_.tile([P, HC], f32, tag="ob")
                        nc.vector.tensor_tensor(
                            ob[:].rearrange("p (c h) -> p c h", c=C),
                            nmp[:, 0:HC].rearrange("p (c h) -> p c h", c=C),
                            rd[:].rearrange("p h -> p () h").to_broadcast(
                                [P, C, H]),
                            op=Alu.mult)
                        sink(wi, ob)

            # ---------------- layer-1 sink: fused layer-2 table build.
            # elu(x) = relu(x) + (exp(-relu(-x)) - 1), Act-heavy form.
            def elu1(ob, pool):
                t0 = pool.tile([P, HC], f32, tag="elu0")
                nc.scalar.activation(t0[:], ob[:], Act.Relu, scale=-1.0)
                nc.scalar.activation(t0[:], t0[:], Act.Exp, scale=-1.0)
                t1 = pool.tile([P, HC], f32, tag="elu1")
                nc.scalar.activation(t1[:], ob[:], Act.Relu)
                return t0, t1

            def sink1(w, ob):
                t0, t1 = elu1(ob, wp)
                nc.vector.tensor_scalar(t0[:], t0[:], -1.0, None,
                                        op0=Alu.add)
                el = wp.tile([P, HC], f32, tag="el1")
                nc.vector.tensor_tensor(el[:], t1[:], t0[:], op=Alu.add)
                ps2 = ppre.tile([P, XC], f32, space="PSUM", tag="ppre")
                for k in range(2):
                    tp = ptp.tile([P, P], f32, space="PSUM", tag="tp")
                    nc.tensor.transpose(tp[:], el[:, k * P:(k + 1) * P],
                                        id_t[:])
                    et = wp.tile([P, P], f32, tag="eT")
                    nc.scalar.copy(et[:], tp[:])
                    nc.tensor.matmul(ps2[:], lhsT=et[:],
                                     rhs=(w2a_t if k == 0 else w2b_t)[:],
                                     start=(k == 0), stop=(k == 1))
                h2 = wp.tile([P, EC], bf, tag="h2")
                nc.vector.tensor_tensor(h2[:, 0:HC], ps2[:, 0:HC],
                                        b2e_t[:, 0:HC], op=Alu.add)
                nc.vector.tensor_tensor(h2[:, HC:EC].bitcast(f32),
                                        ps2[:, HC:XC],
                                        b2e_t[:, HC:XC], op=Alu.add)
                nc.sync.dma_start(T2s[w * P:(w + 1) * P, 0:EC], h2[:])
                if w == NW - 1:
                    cc("AllGather", Alu.bypass,
                       ins=[T2s[:, :]], outs=[T2u[:, :]])

            # -------------------------------- layer-2 sink: mean pooling
            plp = ppl.tile([G, HC + 1], f32, space="PSUM", tag="pool")

            def sink2(w, ob):
                t0, t1 = elu1(ob, op_)
                nc.vector.tensor_scalar(t0[:], t0[:], -1.0, None,
                                        op0=Alu.add)
                el = op_.tile([P, HC + 1], f32, tag="el2")
                nc.vector.tensor_tensor(el[:, 0:HC], t1[:], t0[:],
                                        op=Alu.add)
                nc.vector.memset(el[:, HC:HC + 1], 1.0)
                bm = op_.tile([P, G], f32, tag="bm")
                nc.vector.tensor_scalar(bm[:], io64_t[:],
                                        bat_t[:, w:w + 1], None,
                                        op0=Alu.is_equal)
                nc.tensor.matmul(plp[:], lhsT=bm[:], rhs=el[:],
                                 start=(w == 0), stop=(w == NW - 1))
                if kdbg:
                    nc.sync.dma_start(dbgel[w * P:(w + 1) * P, :],
                                      el[:, 0:HC])
                    nc.sync.dma_start(dbgob[w * P:(w + 1) * P, :], ob[:])

            def epilogue():
                pls = wp.tile([G, HC + 1], f32, tag="pls")
                nc.vector.tensor_copy(pls[:], plp[:])
                nc.sync.dma_start(prd[:, :], pls[:])
                cc("AllReduce", Alu.add, ins=[prd[:, :]], outs=[prs[:, :]])
                pr = wp.tile([G, HC + 1], f32, tag="pr")
                nc.sync.dma_start(pr[:], prs[:, :])
                cnt = wp.tile([G, 1], f32, tag="cnt")
                nc.vector.tensor_scalar(cnt[:], pr[:, HC:HC + 1], 1.0,
                                        None, op0=Alu.max)
                nc.vector.reciprocal(cnt[:], cnt[:])
                pooled = wp.tile([G, HC], f32, tag="pooled")
                nc.vector.tensor_scalar(pooled[:], pr[:, 0:HC],
                                        cnt[:, 0:1], None, op0=Alu.mult)
                psl_full = ppre.tile([P, XC], f32, space="PSUM", tag="ppre")
                psl = psl_full[0:G, 0:2]
                for k in range(2):
                    tp = ptp.tile([P, P], f32, space="PSUM", tag="tpf")
                    nc.tensor.transpose(tp[:, 0:G],
                                        pooled[:, k * P:(k + 1) * P],
                                        id_t[0:G, 0:G])
                    pt = wp.tile([P, G], f32, tag="pT")
                    nc.scalar.copy(pt[:], tp[:, 0:G])
                    nc.tensor.matmul(psl, lhsT=pt[:],
                                     rhs=wl_t[:, 2 * k:2 * k + 2],
                                     start=(k == 0), stop=(k == 1))
                lg = wp.tile([G, 2], f32, tag="lg")
                nc.vector.tensor_tensor(lg[:], psl, bl_t[:], op=Alu.add)
                nc.sync.dma_start(logits[:, :], lg[:])

            phase_a()
            adw1 = panel1(T1u)
            window_loop(T1u, adw1, sink1)
            adw2 = panel1(T2u) if os.environ.get('KP2','0')=='1' else panel2()
            window_loop(T2u, adw2, sink2)
            epilogue()

    nc.compile()
    return nc


def kernel(**inputs):
    from concourse.bass_utils import run_bass_kernel_spmd

    nc, in_maps = prepare(inputs)
    res = run_bass_kernel_spmd(nc, in_maps, core_ids=list(range(NC_)))
    return res.results[0]["logits"]


def prepare(inputs):
    bf16 = _bf16()
    x = np.asarray(inputs["x"], np.float32)
    edge_index = np.asarray(inputs["edge_index"], np.int64)
    batch = np.asarray(inputs["batch"], np.int64)
    W1 = np.asarray(inputs["W1"], np.float32)
    W2 = np.asarray(inputs["W2"], np.float32)
    W_lin = np.asarray(inputs["W_lin"], np.float32)
    b1 = np.asarray(inputs["b1"], np.float32)
    b2 = np.asarray(inputs["b2"], np.float32)
    b_lin = np.asarray(inputs["b_lin"], np.float32)
    a_src1 = np.asarray(inputs["a_src1"], np.float32)
    a_dst1 = np.asarray(inputs["a_dst1"], np.float32)
    a_src2 = np.asarray(inputs["a_src2"], np.float32)
    a_dst2 = np.asarray(inputs["a_dst2"], np.float32)

    has_b1 = bool(np.any(b1))
    meta = _preprocess(edge_index, batch)
    nc = _build(meta, has_b1)

    W1R, b1ext = _fold(W1, a_src1, a_dst1, b1, perm_rows=False)
    W2R, b2ext = _fold(W2, a_src2, a_dst2, b2, perm_rows=True)
    b2eff = b2ext
    wlin_p = W_lin[_PERM]
    blin_eff = b_lin

    iota128 = np.tile(np.arange(P, dtype=np.float32), (P, 1))
    iota64 = np.tile(np.arange(G, dtype=np.float32), (P, 1))
    ident = np.eye(P, dtype=np.float32)
    identB = np.eye(P, dtype=np.float32).astype(bf16)
    wlin_2 = np.concatenate([wlin_p[0:P], wlin_p[P:2 * P]], axis=1)

    # full padded x, transposed: [IN, NP]; same for every core
    xs = np.zeros((P, NP), np.float32)
    xs[:, 0:N] = x.T
    xs = xs.astype(bf16)

    in_maps = []
    for c in range(NC_):
        im = {
            "xT": xs,
            "W1R": W1R.astype(bf16),
            "W2Ra": np.ascontiguousarray(W2R[0:P]),
            "W2Rb": np.ascontiguousarray(W2R[P:2 * P]),
            "b2e": np.tile(b2eff, (P, 1)),
            "Wlin": np.ascontiguousarray(wlin_2),
            "blin": np.tile(blin_eff, (G, 1)),
            "iota128": iota128.astype(bf16),
            "iota64": iota64,
            "ident": ident, "identB": identB,
            "dstloc": np.ascontiguousarray(meta["dstloc"][c]).astype(bf16),
            "stT": np.ascontiguousarray(meta["stT"][c]),
            "idxlo": np.ascontiguousarray(meta["ilo"][c]),
            "idxhi": np.ascontiguousarray(meta["ihi"][c]),
            "idxpanL": np.ascontiguousarray(meta["ipanL"][c]),
            "idxpanH": np.ascontiguousarray(meta["ipanH"][c]),
            "pmask": np.ascontiguousarray(meta["pmask"][c]),
            "batchloc": np.ascontiguousarray(meta["batchloc"][c]),
        }
        if has_b1:
            im["b1e"] = np.tile(b1ext, (P, 1))
        in_maps.append(im)

    return nc, in_maps


# revision 26
# speedup vs baseline: 1.0025x; 1.0025x over previous
"""GATClassifier (2x GATConv + mean-pool + linear) on 8 Trainium2 NeuronCores.

v4: unified-table design.

- No layer-1 AllGather: every core redundantly builds the FULL layer-1 table
  (x@W1 for all 50176 padded nodes is ~100us spread over PE/Act/DVE, far
  cheaper than a 38MB collective).
- ONE layer-2 AllGather of the whole shard table (the collective cost model
  and real fabric both reward one large transfer over two mid-size ones).
  Collectives ride the Pool queue (the only HW-legal bass engine).
- Tables are plain node-major [50176 rows x 384 bf16 cols]; gather tokens are
  split at row 31360 (= 5 shards, int16-safe: lo tokens < 31360, hi tokens
  < 18816) with STATIC in_ap bases, so layer 1 and layer 2 share one token
  stream and one one-hot stream.
- Gathers are merged across 2-window groups (slot layout [w0lo|w1lo|w0hi|
  w1hi], each window-half padded to full 128-slot blocks with junk tokens),
  halving the fixed SWDGE cost per gather.
- a_dst routing: no per-edge 256B gather.  A per-layer "panel" gather pulls
  this core's own 6272 eval blocks (two dynamic-count gathers, one of which
  is a no-op, because the shard lives entirely below or above the 31360
  split); a transposed one-hot (StT, host data) routes a_dst[dst] to edge
  slots with one tiny PE matmul per block.
- h columns are c-major (col = c*H + h) so the per-edge message multiply and
  the softmax divide hit the DVE 2x perf mode (packed 4-wide last dim).
- elu's "-1" is folded into the next layer's bias host-side, and elu is
  computed with two Act Relu/Exp ops + one DVE add.

Everything is SPMD-uniform: all core-specific info arrives as data.
"""

import math
import os

import numpy as np

# ---------------------------------------------------------------- constants
N = 50000       # nodes
E = 800000      # directed edges before self loops
IN = 128        # in channels
H = 4           # heads
C = 64          # channels per head
HC = H * C      # 256
G = 64          # graphs
NC_ = 8         # cores
P = 128
SH = 6272       # padded, 128-aligned nodes per shard (49 windows)
NW = SH // P            # 49 windows per core
NP = NC_ * SH           # 50176 padded nodes (global row == node id)
LSPL = 5 * SH           # 31360 lo/hi token split (shard-aligned, < 32768)
NHI = NP - LSPL         # 18816 hi rows
RW = 384                # bf16 cols per table row (768 B)
XC = HC + 2 * H         # 264 meaningful f32 cols of [h|a_src|a_dst]
EC = HC + 4 * H         # 272 bf16 cols written per row
GW = 2                  # windows per gather group


def _bf16():
    import ml_dtypes
    return ml_dtypes.bfloat16


def _wrap16(tok: np.ndarray) -> np.ndarray:
    """dma_gather index layout: token i lives at [i%16, i//16], replicated
    into all 8 groups of 16 partitions."""
    assert tok.size % 16 == 0
    w = tok.reshape(-1, 16).T.astype(np.int16)  # [16, L/16]
    return np.tile(w, (8, 1))                   # [128, L/16]


def _groups():
    gs, w = [], 0
    while w < NW:
        gs.append(list(range(w, min(w + GW, NW))))
        w += GW
    return gs


def _preprocess(edge_index: np.ndarray, batch: np.ndarray):
    """Host-side integer-only preprocessing: shard edges by dst window, sort
    into (window, lo/hi, src) order, pack into per-group gather streams with
    full-block padding, and emit per-core index/dstloc/one-hot-T arrays."""
    src = np.concatenate([edge_index[0], np.arange(N, dtype=np.int64)])
    dst = np.concatenate([edge_index[1], np.arange(N, dtype=np.int64)])
    half = (src >= LSPL).astype(np.int64)
    tok = np.where(half == 0, src, src - LSPL)

    owner = dst // SH
    dl = dst % SH
    wd, dloc = dl // P, dl % P

    counts = np.zeros((NC_, NW, 2), dtype=np.int64)
    per_core = []
    for c in range(NC_):
        m = owner == c
        t_, h_, w_, d_ = tok[m], half[m], wd[m], dloc[m]
        order = np.lexsort((t_, h_, w_))
        t_, h_, w_, d_ = t_[order], h_[order], w_[order], d_[order]
        np.add.at(counts[c], (w_, h_), 1)
        per_core.append((t_, h_, w_, d_))

    maxcnt = counts.max(axis=0)                       # [NW, 2]
    blo = (maxcnt[:, 0] + P - 1) // P
    bhi = (maxcnt[:, 1] + P - 1) // P
    bw = blo + bhi
    totb = int(bw.sum())

    groups = _groups()
    ng = len(groups)
    # per-group block bases; slot layout per group: [w0lo | w1lo | w0hi | w1hi]
    gbase = np.zeros(ng, dtype=np.int64)
    acc = 0
    for gi, ws in enumerate(groups):
        gbase[gi] = acc
        acc += int(bw[ws].sum())
    assert acc == totb
    bwg = np.array([int(bw[ws].sum()) for ws in groups])
    blog = np.array([int(blo[ws].sum()) for ws in groups])
    bwgmax = int(bwg.max())

    # per-window block ranges (global block ids): (lo_range, hi_range)
    wblocks = [None] * NW
    for gi, ws in enumerate(groups):
        b0 = int(gbase[gi])
        lo0 = b0
        hi0 = b0 + int(blo[ws].sum())
        for wi in ws:
            lor = (lo0, lo0 + int(blo[wi]))
            lo0 = lor[1]
            hir = (hi0, hi0 + int(bhi[wi]))
            hi0 = hir[1]
            wblocks[wi] = (lor, hir)

    dstloc = np.full((NC_, P, totb), -1.0, dtype=np.float32)
    ilo_l, ihi_l = [], []
    for c in range(NC_):
        t_, h_, w_, d_ = per_core[c]
        lo_parts, hi_parts = [], []
        for gi, ws in enumerate(groups):
            for hf, bcnt_arr, parts in ((0, blo, lo_parts),
                                        (1, bhi, hi_parts)):
                for wi in ws:
                    m = (w_ == wi) & (h_ == hf)
                    nreal = int(m.sum())
                    nt = int(bcnt_arr[wi]) * P
                    tt = np.zeros(nt, dtype=np.int64)
                    tt[:nreal] = t_[m]
                    dd = np.full(nt, -1.0, dtype=np.float32)
                    dd[:nreal] = d_[m]
                    parts.append(tt)
                    (lor, hir) = wblocks[wi]
                    b0 = lor[0] if hf == 0 else hir[0]
                    bcnt = int(bcnt_arr[wi])
                    if bcnt:
                        dstloc[c, :, b0:b0 + bcnt] = dd.reshape(bcnt, P).T
        ilo_l.append(np.concatenate([_wrap16(x) for x in lo_parts], axis=1))
        ihi_l.append(np.concatenate([_wrap16(x) for x in hi_parts], axis=1))
    ilo = np.stack(ilo_l)
    ihi = np.stack(ihi_l)

    # per-group offsets (in 8-col = 16-token units) into ilo/ihi
    ntlo_g = np.array([int(blo[ws].sum()) * P for ws in groups])
    nthi_g = np.array([int(bhi[ws].sum()) * P for ws in groups])
    lo_off8 = np.concatenate([[0], np.cumsum(ntlo_g // 16)[:-1]]).astype(
        np.int64)
    hi_off8 = np.concatenate([[0], np.cumsum(nthi_g // 16)[:-1]]).astype(
        np.int64)

    # transposed one-hot StT[j, b*128+s] = (dstloc[s, b] == j), bf16
    bf16 = _bf16()
    stT = np.zeros((NC_, P, totb * P), dtype=bf16)
    for c in range(NC_):
        dl_ = dstloc[c]                       # [s=128, b=totb]
        s_idx, b_idx = np.nonzero(dl_ >= 0.0)
        j_idx = dl_[s_idx, b_idx].astype(np.int64)
        stT[c][j_idx, b_idx * P + s_idx] = 1.0

    # both panel gathers are fully static: the inactive side gathers SH
    # safe junk rows (masked out later); no dynamic counts anywhere
    junk = np.arange(SH, dtype=np.int64)
    ipanL = np.zeros((NC_, P, SH // 16), dtype=np.int16)
    ipanH = np.zeros((NC_, P, SH // 16), dtype=np.int16)
    pmask = np.zeros((NC_, P, 2), dtype=np.float32)
    for c in range(NC_):
        rows = np.arange(c * SH, (c + 1) * SH, dtype=np.int64)
        if c * SH >= LSPL:
            ipanH[c] = _wrap16(rows - LSPL)
            ipanL[c] = _wrap16(junk)
            pmask[c, :, 1] = 1.0
        else:
            ipanL[c] = _wrap16(rows)
            ipanH[c] = _wrap16(junk)
            pmask[c, :, 0] = 1.0

    # batch (graph id) per local node slot; -1 on ghost slots
    batchloc = np.full((NC_, P, NW), -1.0, dtype=np.float32)
    for c in range(NC_):
        lo, hi = c * SH, min((c + 1) * SH, N)
        b = np.full(SH, -1.0, dtype=np.float32)
        if hi > lo:
            b[:hi - lo] = batch[lo:hi].astype(np.float32)
        batchloc[c] = b.reshape(NW, P).T

    return dict(
        blo=blo.astype(int), bhi=bhi.astype(int), bw=bw.astype(int),
        totb=totb, groups=groups, gbase=gbase, bwg=bwg, blog=blog,
        bwgmax=bwgmax, wblocks=wblocks,
        ntlo_g=ntlo_g, nthi_g=nthi_g, lo_off8=lo_off8, hi_off8=hi_off8,
        ilo=ilo, ihi=ihi, stT=stT, ipanL=ipanL, ipanH=ipanH, pmask=pmask,
        dstloc=dstloc, batchloc=batchloc,
    )


# c-major permutation: new col c*H+h holds original col h*C+c
_PERM = np.array([h * C + c for c in range(C) for h in range(H)], np.int64)


def _fold(Wm, a_s, a_d, b, perm_rows: bool):
    """[W(c-major cols) | A_src | A_dst] and matching extended bias."""
    K = Wm.shape[0]
    As = np.einsum("khc,hc->kh", Wm.reshape(K, H, C), a_s)
    Ad = np.einsum("khc,hc->kh", Wm.reshape(K, H, C), a_d)
    WR = np.concatenate([Wm[:, _PERM], As, Ad], axis=1).astype(np.float32)
    if perm_rows:
        WR = WR[_PERM]
    be = np.concatenate(
        [b[_PERM], np.einsum("hc,hc->h", b.reshape(H, C), a_s),
         np.einsum("hc,hc->h", b.reshape(H, C), a_d)]
    ).astype(np.float32)                                           # [264]
    return WR, be


def _build(meta, has_b1: bool):
    import concourse.bacc as bacc
    import concourse.mybir as mybir
    import concourse.tile as tile

    kq = int(os.environ.get("KQ", "1"))        # swdge queues
    kbg = int(os.environ.get("KBG", "2"))      # gather pool bufs

    f32 = mybir.dt.float32
    bf = mybir.dt.bfloat16
    i16 = mybir.dt.int16
    i32 = mybir.dt.int32
    Act = mybir.ActivationFunctionType
    Alu = mybir.AluOpType

    groups = meta["groups"]
    blog, bwg, gbase = meta["blog"], meta["bwg"], meta["gbase"]
    ntlo_g, nthi_g = meta["ntlo_g"], meta["nthi_g"]
    lo_off8, hi_off8 = meta["lo_off8"], meta["hi_off8"]
    wblocks = meta["wblocks"]
    TOTB, BWG = meta["totb"], meta["bwgmax"]
    MBW = int(meta["bw"].max())
    NLO8, NHI8 = int((ntlo_g // 16).sum()), int((nthi_g // 16).sum())

    nc = bacc.Bacc("TRN2", target_bir_lowering=False, debug=False,
                   num_devices=NC_, num_swdge_queues=kq)

    grp = [list(range(NC_))]

    def cc(kind, op, ins, outs):
        nc.gpsimd.collective_compute(
            kind, op, replica_groups=grp, ins=ins, outs=outs)

    # ------------------------------------------------------------- tensors
    xT = nc.dram_tensor("xT", [P, NP], bf, kind="ExternalInput")
    W1R = nc.dram_tensor("W1R", [IN, XC], bf, kind="ExternalInput")
    W2Ra = nc.dram_tensor("W2Ra", [P, XC], f32, kind="ExternalInput")
    W2Rb = nc.dram_tensor("W2Rb", [P, XC], f32, kind="ExternalInput")
    b2e = nc.dram_tensor("b2e", [P, XC], f32, kind="ExternalInput")
    Wlin = nc.dram_tensor("Wlin", [P, 4], f32, kind="ExternalInput")
    blin = nc.dram_tensor("blin", [G, 2], f32, kind="ExternalInput")
    iota128 = nc.dram_tensor("iota128", [P, P], bf, kind="ExternalInput")
    iota64 = nc.dram_tensor("iota64", [P, G], f32, kind="ExternalInput")
    ident = nc.dram_tensor("ident", [P, P], f32, kind="ExternalInput")
    identB = nc.dram_tensor("identB", [P, P], bf, kind="ExternalInput")
    dstloc = nc.dram_tensor("dstloc", [P, TOTB], bf, kind="ExternalInput")
    stT = nc.dram_tensor("stT", [P, TOTB * P], bf, kind="ExternalInput")
    idxlo = nc.dram_tensor("idxlo", [P, NLO8], i16, kind="ExternalInput")
    idxhi = nc.dram_tensor("idxhi", [P, NHI8], i16, kind="ExternalInput")
    idxpanL = nc.dram_tensor("idxpanL", [P, SH // 16], i16,
                             kind="ExternalInput")
    idxpanH = nc.dram_tensor("idxpanH", [P, SH // 16], i16,
                             kind="ExternalInput")
    pmask = nc.dram_tensor("pmask", [P, 2], f32, kind="ExternalInput")
    batchloc = nc.dram_tensor("batchloc", [P, NW], f32, kind="ExternalInput")
    if has_b1:
        b1e = nc.dram_tensor("b1e", [P, XC], f32, kind="ExternalInput")

    logits = nc.dram_tensor("logits", [G, 2], f32, kind="ExternalOutput")
    kdbg = os.environ.get("KDBG", "0") == "1"
    if kdbg:
        dbgel = nc.dram_tensor("dbgel", [SH, HC], f32, kind="ExternalOutput")
        dbgob = nc.dram_tensor("dbgob", [SH, HC], f32, kind="ExternalOutput")

    T1u = nc.dram_tensor("T1u", [NP, RW], bf, kind="Internal")
    T2s = nc.dram_tensor("T2s", [SH, RW], bf, kind="Internal")
    T2u = nc.dram_tensor("T2u", [NP, RW], bf, kind="Internal",
                         addr_space="Shared")
    prd = nc.dram_tensor("prd", [G, HC + 1], f32, kind="Internal")
    prs = nc.dram_tensor("prs", [G, HC + 1], f32, kind="Internal",
                         addr_space="Shared")

    with tile.TileContext(nc) as tc:
        with (
            tc.tile_pool(name="const", bufs=1) as cp,
            tc.tile_pool(name="work", bufs=3) as wp,
            tc.tile_pool(name="xw", bufs=2) as xp,
            tc.tile_pool(name="gat", bufs=kbg) as gp,
            tc.tile_pool(name="sel", bufs=2) as sp,
            tc.tile_pool(name="selt", bufs=2) as stp,
            tc.tile_pool(name="pan", bufs=1) as pnp,
            tc.tile_pool(name="adw", bufs=4) as awp,
            tc.tile_pool(name="msg", bufs=2) as mp,
            tc.tile_pool(name="outp", bufs=2) as op_,
            tc.tile_pool(name="ppre", bufs=2, space="PSUM") as ppre,
            tc.tile_pool(name="ptp", bufs=1, space="PSUM") as ptp,
            tc.tile_pool(name="pnum", bufs=2, space="PSUM") as pnum,
            tc.tile_pool(name="ppool", bufs=1, space="PSUM") as ppl,
            tc.tile_pool(name="pad", bufs=1, space="PSUM") as pap,
        ):
            # ---------------------------------------------------- constants
            def cload(dram, dt):
                tl = cp.tile(list(dram.shape), dt, tag=dram.name)
                nc.sync.dma_start(tl[:], dram[:])
                return tl

            w1r_t = cload(W1R, bf)
            w2a_t = cload(W2Ra, f32)
            w2b_t = cload(W2Rb, f32)
            b2e_t = cload(b2e, f32)
            wl_t = cload(Wlin, f32)
            bl_t = cload(blin, f32)
            io64_t = cload(iota64, f32)
            id_t = cload(ident, f32)
            idB_t = cload(identB, bf)
            ilo_t = cload(idxlo, i16)
            ihi_t = cload(idxhi, i16)
            ipanL_t = cload(idxpanL, i16)
            ipanH_t = cload(idxpanH, i16)
            pm_t = cload(pmask, f32)
            bat_t = cload(batchloc, f32)
            if has_b1:
                b1e_t = cload(b1e, f32)
            io1_t = cp.tile([P, 1, P], bf, tag="io1")
            nc.sync.dma_start(io1_t[:, 0, :], iota128[:])
            dst3_t = cp.tile([P, TOTB, 1], bf, tag="dst3")
            nc.sync.dma_start(
                dst3_t[:].rearrange("p b one -> p (b one)"), dstloc[:])

            # --------------------------------------------- layer-1 table
            # (full, built redundantly on every core, in global row order)
            def phase_a():
                nb = math.ceil(NP // P / 4)
                for bi in range(nb):
                    w0 = bi * 4
                    n = min(4, NP // P - w0)
                    g0 = w0 * P
                    xt = xp.tile([P, 4 * P], bf, tag="xt")
                    nc.sync.dma_start(xt[:, 0:n * P], xT[:, g0:g0 + n * P])
                    h4 = xp.tile([P, 4, EC], bf, tag="h4")
                    for k in range(n):
                        ps = ppre.tile([P, XC], f32, space="PSUM", tag="ppre")
                        nc.tensor.matmul(ps[:], lhsT=xt[:, k * P:(k + 1) * P],
                                         rhs=w1r_t[:], start=True, stop=True)
                        if has_b1:
                            nc.vector.tensor_tensor(
                                h4[:, k, 0:HC], ps[:, 0:HC],
                                b1e_t[:, 0:HC], op=Alu.add)
                            nc.vector.tensor_tensor(
                                h4[:, k, HC:EC].bitcast(f32),
                                ps[:, HC:XC], b1e_t[:, HC:XC], op=Alu.add)
                        else:
                            eng = (nc.scalar.copy, nc.vector.tensor_copy
                                   )[(bi * 4 + k) % 2]
                            eng(h4[:, k, 0:HC], ps[:, 0:HC])
                            nc.vector.tensor_copy(
                                h4[:, k, HC:EC].bitcast(f32), ps[:, HC:XC])
                    nc.sync.dma_start(
                        T1u[g0:g0 + n * P, 0:EC].rearrange(
                            "(k p) e -> p k e", p=P),
                        h4[:, 0:n, :])

            # ------------------------------------------------ panel gather
            # own shard's eval blocks -> a_dst[j] per window, [P, NW, H] bf16
            NWH = 25                      # panel windows per pass
            SHH = NWH * P                 # 3200 tokens per pass

            def panel1(Tu):
                adw = awp.tile([P, NW, H], bf, tag="adw")
                # each pass covers up to NWH windows; the inactive side
                # gathers 16 junk tokens per pass and is masked out
                for t0 in range(0, NW, NWH):
                    nwp = min(NWH, NW - t0)
                    ntp = nwp * P
                    ptL = pnp.tile([P, NWH, P], bf, tag="panL")
                    nc.gpsimd.dma_gather(
                        ptL[:, 0:nwp, :], Tu[0:LSPL, HC:HC + P],
                        ipanL_t[:, t0 * 8:(t0 + nwp) * 8],
                        ntp, ntp, P, elem_step=RW, single_packet=False)
                    ptH = pnp.tile([P, NWH, P], bf, tag="panH")
                    nc.gpsimd.dma_gather(
                        ptH[:, 0:nwp, :], Tu[LSPL:NP, HC:HC + P],
                        ipanH_t[:, t0 * 8:(t0 + nwp) * 8],
                        ntp, ntp, P, elem_step=RW, single_packet=False,
                        queue_num=1 % kq)
                    aw = awp.tile([P, NWH, H], f32, tag="aw32")
                    nc.vector.tensor_scalar(
                        aw[:, 0:nwp, :],
                        ptL[:, 0:nwp, :].bitcast(f32)[:, :, H:2 * H],
                        pm_t[:, 0:1], None, op0=Alu.mult)
                    ah = awp.tile([P, NWH, H], f32, tag="ah32")
                    nc.vector.tensor_scalar(
                        ah[:, 0:nwp, :],
                        ptH[:, 0:nwp, :].bitcast(f32)[:, :, H:2 * H],
                        pm_t[:, 1:2], None, op0=Alu.mult)
                    nc.vector.tensor_tensor(adw[:, t0:t0 + nwp, :],
                                            aw[:, 0:nwp, :],
                                            ah[:, 0:nwp, :], op=Alu.add)
                return adw

            def panel2():
                # T2s is core-local: the eval blocks are a plain DMA away
                adw = awp.tile([P, NW, H], bf, tag="adw")
                for t0 in range(0, NW, NWH):
                    nwp = min(NWH, NW - t0)
                    pt = pnp.tile([P, NWH, P], bf, tag="panL")
                    nc.sync.dma_start(
                        pt[:, 0:nwp, :],
                        T2s[t0 * P:(t0 + nwp) * P, HC:HC + P].rearrange(
                            "(w p) c -> p w c", p=P))
                    nc.vector.tensor_copy(
                        adw[:, t0:t0 + nwp, :],
                        pt[:, 0:nwp, :].bitcast(f32)[:, :, H:2 * H])
                return adw

            # shared window loop -----------------------------------------
            def window_loop(Tu, adw, sink):
                for gi, ws in enumerate(groups):
                    BL, BW = int(blog[gi]), int(bwg[gi])
                    b0 = int(gbase[gi])
                    NTL, NTH = int(ntlo_g[gi]), int(nthi_g[gi])
                    Gt = gp.tile([P, BWG, RW], bf, tag="G")
                    nc.gpsimd.dma_gather(
                        Gt[:, 0:BL, :], Tu[0:LSPL, :],
                        ilo_t[:, int(lo_off8[gi]):
                              int(lo_off8[gi]) + NTL // 16],
                        NTL, NTL, RW, single_packet=False)
                    nc.gpsimd.dma_gather(
                        Gt[:, BL:BW, :], Tu[LSPL:NP, :],
                        ihi_t[:, int(hi_off8[gi]):
                              int(hi_off8[gi]) + NTH // 16],
                        NTH, NTH, RW, single_packet=False,
                        queue_num=1 % kq)

                    # one-hot: S[p, b, j] = (j == dstloc[p, b]); and its
                    # transpose StT (host data) for a_dst routing
                    St = sp.tile([P, BWG, P], bf, tag="S")
                    nc.vector.tensor_tensor(
                        St[:, :BW, :],
                        io1_t[:].to_broadcast([P, BW, P]),
                        dst3_t[:, b0:b0 + BW, :].to_broadcast([P, BW, P]),
                        op=Alu.is_equal)
                    StT = stp.tile([P, BWG, P], bf, tag="ST")
                    nc.sync.dma_start(
                        StT[:, 0:BW, :].rearrange("p b s -> p (b s)"),
                        stT[:, b0 * P:(b0 + BW) * P])

                    # a_dst per edge slot: adp[s, b, h] = sum_j StT[j, b, s]
                    # * adw[j, w(b), h]  (tiny per-block PE matmuls)
                    adp = pap.tile([P, BWG, H], f32, space="PSUM", tag="ad")
                    for wi in ws:
                        for r0, r1 in wblocks[wi]:
                            for b in range(r0, r1):
                                nc.tensor.matmul(
                                    adp[:, b - b0, :],
                                    lhsT=StT[:, b - b0, :],
                                    rhs=adw[:, wi, :],
                                    start=True, stop=True)

                    # e = a_src(gathered) + a_dst(routed);
                    # exp(leaky(e)) as exp(0.6*(e + (2/3)|e|))
                    ev = mp.tile([P, BWG, H], f32, tag="ev")
                    GtF = Gt[:].bitcast(f32)
                    nc.vector.tensor_tensor(ev[:, :BW, :],
                                            GtF[:, :BW, P:P + H],
                                            adp[:, :BW, :], op=Alu.add)
                    av = mp.tile([P, BWG, H], f32, tag="av")
                    nc.scalar.activation(av[:, :BW, :], ev[:, :BW, :],
                                         Act.Abs, scale=2.0 / 3.0)
                    nc.vector.tensor_tensor(av[:, :BW, :], ev[:, :BW, :],
                                            av[:, :BW, :], op=Alu.add)
                    nc.vector.tensor_scalar(av[:, :BW, :], av[:, :BW, :],
                                            60.0, None, op0=Alu.min)
                    ex = mp.tile([P, BWG, H], bf, tag="ex")
                    nc.scalar.activation(ex[:, :BW, :], av[:, :BW, :],
                                         Act.Exp, scale=0.6)

                    for wi in ws:
                        # msg = [h * ex | ex]; c-major h -> packed 4-wide
                        # last dim for the DVE fast mode
                        (lor, hir) = wblocks[wi]
                        blocks = list(range(*lor)) + list(range(*hir))
                        nb_w = len(blocks)
                        msg = mp.tile([P, MBW, HC + H], bf, tag="msg")
                        for j0, (r0, r1) in ((0, lor),
                                             (lor[1] - lor[0], hir)):
                            if r1 == r0:
                                continue
                            nbr = r1 - r0
                            nc.vector.tensor_tensor(
                                msg[:, j0:j0 + nbr, 0:HC].rearrange(
                                    "p b (c h) -> p b c h", c=C),
                                Gt[:, r0 - b0:r1 - b0, 0:HC].rearrange(
                                    "p b (c h) -> p b c h", c=C),
                                ex[:, r0 - b0:r1 - b0, :].rearrange(
                                    "p b h -> p b () h").to_broadcast(
                                        [P, nbr, C, H]),
                                op=Alu.mult)
                            nc.scalar.copy(
                                msg[:, j0:j0 + nbr, HC:HC + H],
                                ex[:, r0 - b0:r1 - b0, :])
                        nmp = pnum.tile([P, HC + H], f32, space="PSUM",
                                        tag="nm")
                        for i, b in enumerate(blocks):
                            nc.tensor.matmul(nmp[:], lhsT=St[:, b - b0, :],
                                             rhs=msg[:, i, 0:HC + H],
                                             start=(i == 0),
                                             stop=(i == nb_w - 1))
                        rd = mp.tile([P, H], f32, tag="rd")
                        nc.vector.tensor_scalar(rd[:], nmp[:, HC:HC + H],
                                                1e-30, None, op0=Alu.max)
                        nc.vector.reciprocal(rd[:], rd[:])
                        ob = op_.tile([P, HC], f32, tag="ob")
                        nc.vector.tensor_tensor(
                            ob[:].rearrange("p (c h) -> p c h", c=C),
                            nmp[:, 0:HC].rearrange("p (c h) -> p c h", c=C),
                            rd[:].rearrange("p h -> p () h").to_broadcast(
                                [P, C, H]),
                            op=Alu.mult)
                        sink(wi, ob)

            # ---------------- layer-1 sink: fused layer-2 table build.
            # elu(x) = relu(x) + (exp(-relu(-x)) - 1), Act-heavy form.
            def elu1(ob, pool):
                t0 = pool.tile([P, HC], f32, tag="elu0")
                nc.scalar.activation(t0[:], ob[:], Act.Relu, scale=-1.0)
                nc.scalar.activation(t0[:], t0[:], Act.Exp, scale=-1.0)
                t1 = pool.tile([P, HC], f32, tag="elu1")
                nc.scalar.activation(t1[:], ob[:], Act.Relu)
                return t0, t1

            def sink1(w, ob):
                t0, t1 = elu1(ob, wp)
                nc.vector.tensor_scalar(t0[:], t0[:], -1.0, None,
                                        op0=Alu.add)
                el = wp.tile([P, HC], f32, tag="el1")
                nc.vector.tensor_tensor(el[:], t1[:], t0[:], op=Alu.add)
                ps2 = ppre.tile([P, XC], f32, space="PSUM", tag="ppre")
                for k in range(2):
                    tp = ptp.tile([P, P], f32, space="PSUM", tag="tp")
                    nc.tensor.transpose(tp[:], el[:, k * P:(k + 1) * P],
                                        id_t[:])
                    et = wp.tile([P, P], f32, tag="eT")
                    nc.scalar.copy(et[:], tp[:])
                    nc.tensor.matmul(ps2[:], lhsT=et[:],
                                     rhs=(w2a_t if k == 0 else w2b_t)[:],
                                     start=(k == 0), stop=(k == 1))
                h2 = wp.tile([P, EC], bf, tag="h2")
                nc.vector.tensor_tensor(h2[:, 0:HC], ps2[:, 0:HC],
                                        b2e_t[:, 0:HC], op=Alu.add)
                nc.vector.tensor_tensor(h2[:, HC:EC].bitcast(f32),
                                        ps2[:, HC:XC],
                                        b2e_t[:, HC:XC], op=Alu.add)
                nc.sync.dma_start(T2s[w * P:(w + 1) * P, 0:EC], h2[:])
                if w == NW - 1:
                    cc("AllGather", Alu.bypass,
                       ins=[T2s[:, :]], outs=[T2u[:, :]])

            # -------------------------------- layer-2 sink: mean pooling
            plp = ppl.tile([G, HC + 1], f32, space="PSUM", tag="pool")

            def sink2(w, ob):
                t0, t1 = elu1(ob, op_)
                nc.vector.tensor_scalar(t0[:], t0[:], -1.0, None,
                                        op0=Alu.add)
                el = op_.tile([P, HC + 1], f32, tag="el2")
                nc.vector.tensor_tensor(el[:, 0:HC], t1[:], t0[:],
                                        op=Alu.add)
                nc.vector.memset(el[:, HC:HC + 1], 1.0)
                bm = op_.tile([P, G], f32, tag="bm")
                nc.vector.tensor_scalar(bm[:], io64_t[:],
                                        bat_t[:, w:w + 1], None,
                                        op0=Alu.is_equal)
                nc.tensor.matmul(plp[:], lhsT=bm[:], rhs=el[:],
                                 start=(w == 0), stop=(w == NW - 1))
                if kdbg:
                    nc.sync.dma_start(dbgel[w * P:(w + 1) * P, :],
                                      el[:, 0:HC])
                    nc.sync.dma_start(dbgob[w * P:(w + 1) * P, :], ob[:])

            def epilogue():
                pls = wp.tile([G, HC + 1], f32, tag="pls")
                nc.vector.tensor_copy(pls[:], plp[:])
                nc.sync.dma_start(prd[:, :], pls[:])
                cc("AllReduce", Alu.add, ins=[prd[:, :]], outs=[prs[:, :]])
                pr = wp.tile([G, HC + 1], f32, tag="pr")
                nc.sync.dma_start(pr[:], prs[:, :])
                cnt = wp.tile([G, 1], f32, tag="cnt")
                nc.vector.tensor_scalar(cnt[:], pr[:, HC:HC + 1], 1.0,
                                        None, op0=Alu.max)
                nc.vector.reciprocal(cnt[:], cnt[:])
                pooled = wp.tile([G, HC], f32, tag="pooled")
                nc.vector.tensor_scalar(pooled[:], pr[:, 0:HC],
                                        cnt[:, 0:1], None, op0=Alu.mult)
                psl_full = ppre.tile([P, XC], f32, space="PSUM", tag="ppre")
                psl = psl_full[0:G, 0:2]
                for k in range(2):
                    tp = ptp.tile([P, P], f32, space="PSUM", tag="tpf")
                    nc.tensor.transpose(tp[:, 0:G],
                                        pooled[:, k * P:(k + 1) * P],
                                        id_t[0:G, 0:G])
                    pt = wp.tile([P, G], f32, tag="pT")
                    nc.scalar.copy(pt[:], tp[:, 0:G])
                    nc.tensor.matmul(psl, lhsT=pt[:],
                                     rhs=wl_t[:, 2 * k:2 * k + 2],
                                     start=(k == 0), stop=(k == 1))
                lg = wp.tile([G, 2], f32, tag="lg")
                nc.vector.tensor_tensor(lg[:], psl, bl_t[:], op=Alu.add)
                nc.sync.dma_start(logits[:, :], lg[:])

            phase_a()
            adw1 = panel1(T1u)
            window_loop(T1u, adw1, sink1)
            adw2 = panel1(T2u) if os.environ.get('KP2','0')=='1' else panel2()
            window_loop(T2u, adw2, sink2)
            epilogue()

    nc.compile()
    return nc


def kernel(**inputs):
    from concourse.bass_utils import run_bass_kernel_spmd

    nc, in_maps = prepare(inputs)
    res = run_bass_kernel_spmd(nc, in_maps, core_ids=list(range(NC_)))
    return res.results[0]["logits"]


def prepare(inputs):
    bf16 = _bf16()
    x = np.asarray(inputs["x"], np.float32)
    edge_index = np.asarray(inputs["edge_index"], np.int64)
    batch = np.asarray(inputs["batch"], np.int64)
    W1 = np.asarray(inputs["W1"], np.float32)
    W2 = np.asarray(inputs["W2"], np.float32)
    W_lin = np.asarray(inputs["W_lin"], np.float32)
    b1 = np.asarray(inputs["b1"], np.float32)
    b2 = np.asarray(inputs["b2"], np.float32)
    b_lin = np.asarray(inputs["b_lin"], np.float32)
    a_src1 = np.asarray(inputs["a_src1"], np.float32)
    a_dst1 = np.asarray(inputs["a_dst1"], np.float32)
    a_src2 = np.asarray(inputs["a_src2"], np.float32)
    a_dst2 = np.asarray(inputs["a_dst2"], np.float32)

    has_b1 = bool(np.any(b1))
    meta = _preprocess(edge_index, batch)
    nc = _build(meta, has_b1)

    W1R, b1ext = _fold(W1, a_src1, a_dst1, b1, perm_rows=False)
    W2R, b2ext = _fold(W2, a_src2, a_dst2, b2, perm_rows=True)
    b2eff = b2ext
    wlin_p = W_lin[_PERM]
    blin_eff = b_lin

    iota128 = np.tile(np.arange(P, dtype=np.float32), (P, 1))
    iota64 = np.tile(np.arange(G, dtype=np.float32), (P, 1))
    ident = np.eye(P, dtype=np.float32)
    identB = np.eye(P, dtype=np.float32).astype(bf16)
    wlin_2 = np.concatenate([wlin_p[0:P], wlin_p[P:2 * P]], axis=1)

    # full padded x, transposed: [IN, NP]; same for every core
    xs = np.zeros((P, NP), np.float32)
    xs[:, 0:N] = x.T
    xs = xs.astype(bf16)

    in_maps = []
    for c in range(NC_):
        im = {
            "xT": xs,
            "W1R": W1R.astype(bf16),
            "W2Ra": np.ascontiguousarray(W2R[0:P]),
            "W2Rb": np.ascontiguousarray(W2R[P:2 * P]),
            "b2e": np.tile(b2eff, (P, 1)),
            "Wlin": np.ascontiguousarray(wlin_2),
            "blin": np.tile(blin_eff, (G, 1)),
            "iota128": iota128.astype(bf16),
            "iota64": iota64,
            "ident": ident, "identB": identB,
            "dstloc": np.ascontiguousarray(meta["dstloc"][c]).astype(bf16),
            "stT": np.ascontiguousarray(meta["stT"][c]),
            "idxlo": np.ascontiguousarray(meta["ilo"][c]),
            "idxhi": np.ascontiguousarray(meta["ihi"][c]),
            "idxpanL": np.ascontiguousarray(meta["ipanL"][c]),
            "idxpanH": np.ascontiguousarray(meta["ipanH"][c]),
            "pmask": np.ascontiguousarray(meta["pmask"][c]),
            "batchloc": np.ascontiguousarray(meta["batchloc"][c]),
        }
        if has_b1:
            im["b1e"] = np.tile(b1ext, (P, 1))
        in_maps.append(im)

    return nc, in_maps


# revision 28
# speedup vs baseline: 1.1460x; 1.1431x over previous
"""GATClassifier (2x GATConv + mean-pool + linear) on 8 Trainium2 NeuronCores.

v4: unified-table design.

- No layer-1 AllGather: every core redundantly builds the FULL layer-1 table
  (x@W1 for all 50176 padded nodes is ~100us spread over PE/Act/DVE, far
  cheaper than a 38MB collective).
- ONE layer-2 AllGather of the whole shard table (the collective cost model
  and real fabric both reward one large transfer over two mid-size ones).
  Collectives ride the Pool queue (the only HW-legal bass engine).
- Tables are plain node-major [50176 rows x 384 bf16 cols]; gather tokens are
  split at row 31360 (= 5 shards, int16-safe: lo tokens < 31360, hi tokens
  < 18816) with STATIC in_ap bases, so layer 1 and layer 2 share one token
  stream and one one-hot stream.
- Gathers are merged across 2-window groups (slot layout [w0lo|w1lo|w0hi|
  w1hi], each window-half padded to full 128-slot blocks with junk tokens),
  halving the fixed SWDGE cost per gather.
- a_dst routing: no per-edge 256B gather.  A per-layer "panel" gather pulls
  this core's own 6272 eval blocks (two dynamic-count gathers, one of which
  is a no-op, because the shard lives entirely below or above the 31360
  split); a transposed one-hot (StT, host data) routes a_dst[dst] to edge
  slots with one tiny PE matmul per block.
- h columns are c-major (col = c*H + h) so the per-edge message multiply and
  the softmax divide hit the DVE 2x perf mode (packed 4-wide last dim).
- elu's "-1" is folded into the next layer's bias host-side, and elu is
  computed with two Act Relu/Exp ops + one DVE add.

Everything is SPMD-uniform: all core-specific info arrives as data.
"""

import math
import os

import numpy as np

# ---------------------------------------------------------------- constants
N = 50000       # nodes
E = 800000      # directed edges before self loops
IN = 128        # in channels
H = 4           # heads
C = 64          # channels per head
HC = H * C      # 256
G = 64          # graphs
NC_ = 8         # cores
P = 128
SH = 6272       # padded, 128-aligned nodes per shard (49 windows)
NW = SH // P            # 49 windows per core
NP = NC_ * SH           # 50176 padded nodes (global row == node id)
LSPL = 5 * SH           # 31360 lo/hi token split (shard-aligned, < 32768)
NHI = NP - LSPL         # 18816 hi rows
RW = 384                # bf16 cols per table row (768 B)
XC = HC + 2 * H         # 264 meaningful f32 cols of [h|a_src|a_dst]
EC = HC + 4 * H         # 272 bf16 cols written per row
GW = 2                  # windows per gather group


def _bf16():
    import ml_dtypes
    return ml_dtypes.bfloat16


def _wrap16(tok: np.ndarray) -> np.ndarray:
    """dma_gather index layout: token i lives at [i%16, i//16], replicated
    into all 8 groups of 16 partitions."""
    assert tok.size % 16 == 0
    w = tok.reshape(-1, 16).T.astype(np.int16)  # [16, L/16]
    return np.tile(w, (8, 1))                   # [128, L/16]


def _groups():
    gs, w = [], 0
    while w < NW:
        gs.append(list(range(w, min(w + GW, NW))))
        w += GW
    return gs


def _preprocess(edge_index: np.ndarray, batch: np.ndarray):
    """Host-side integer-only preprocessing: shard edges by dst window, sort
    into (window, lo/hi, src) order, pack into per-group gather streams with
    full-block padding, and emit per-core index/dstloc/one-hot-T arrays."""
    src = np.concatenate([edge_index[0], np.arange(N, dtype=np.int64)])
    dst = np.concatenate([edge_index[1], np.arange(N, dtype=np.int64)])
    half = (src >= LSPL).astype(np.int64)
    tok = np.where(half == 0, src, src - LSPL)

    owner = dst // SH
    dl = dst % SH
    wd, dloc = dl // P, dl % P

    counts = np.zeros((NC_, NW, 2), dtype=np.int64)
    per_core = []
    for c in range(NC_):
        m = owner == c
        t_, h_, w_, d_ = tok[m], half[m], wd[m], dloc[m]
        order = np.lexsort((t_, h_, w_))
        t_, h_, w_, d_ = t_[order], h_[order], w_[order], d_[order]
        np.add.at(counts[c], (w_, h_), 1)
        per_core.append((t_, h_, w_, d_))

    maxcnt = counts.max(axis=0)                       # [NW, 2]
    blo = (maxcnt[:, 0] + P - 1) // P
    bhi = (maxcnt[:, 1] + P - 1) // P
    bw = blo + bhi
    totb = int(bw.sum())

    groups = _groups()
    ng = len(groups)
    # per-group block bases; slot layout per group: [w0lo | w1lo | w0hi | w1hi]
    gbase = np.zeros(ng, dtype=np.int64)
    acc = 0
    for gi, ws in enumerate(groups):
        gbase[gi] = acc
        acc += int(bw[ws].sum())
    assert acc == totb
    bwg = np.array([int(bw[ws].sum()) for ws in groups])
    blog = np.array([int(blo[ws].sum()) for ws in groups])
    bwgmax = int(bwg.max())

    # per-window block ranges (global block ids): (lo_range, hi_range)
    wblocks = [None] * NW
    for gi, ws in enumerate(groups):
        b0 = int(gbase[gi])
        lo0 = b0
        hi0 = b0 + int(blo[ws].sum())
        for wi in ws:
            lor = (lo0, lo0 + int(blo[wi]))
            lo0 = lor[1]
            hir = (hi0, hi0 + int(bhi[wi]))
            hi0 = hir[1]
            wblocks[wi] = (lor, hir)

    dstloc = np.full((NC_, P, totb), -1.0, dtype=np.float32)
    ilo_l, ihi_l = [], []
    for c in range(NC_):
        t_, h_, w_, d_ = per_core[c]
        lo_parts, hi_parts = [], []
        for gi, ws in enumerate(groups):
            for hf, bcnt_arr, parts in ((0, blo, lo_parts),
                                        (1, bhi, hi_parts)):
                for wi in ws:
                    m = (w_ == wi) & (h_ == hf)
                    nreal = int(m.sum())
                    nt = int(bcnt_arr[wi]) * P
                    tt = np.zeros(nt, dtype=np.int64)
                    tt[:nreal] = t_[m]
                    dd = np.full(nt, -1.0, dtype=np.float32)
                    dd[:nreal] = d_[m]
                    parts.append(tt)
                    (lor, hir) = wblocks[wi]
                    b0 = lor[0] if hf == 0 else hir[0]
                    bcnt = int(bcnt_arr[wi])
                    if bcnt:
                        dstloc[c, :, b0:b0 + bcnt] = dd.reshape(bcnt, P).T
        ilo_l.append(np.concatenate([_wrap16(x) for x in lo_parts], axis=1))
        ihi_l.append(np.concatenate([_wrap16(x) for x in hi_parts], axis=1))
    ilo = np.stack(ilo_l)
    ihi = np.stack(ihi_l)

    # per-group offsets (in 8-col = 16-token units) into ilo/ihi
    ntlo_g = np.array([int(blo[ws].sum()) * P for ws in groups])
    nthi_g = np.array([int(bhi[ws].sum()) * P for ws in groups])
    lo_off8 = np.concatenate([[0], np.cumsum(ntlo_g // 16)[:-1]]).astype(
        np.int64)
    hi_off8 = np.concatenate([[0], np.cumsum(nthi_g // 16)[:-1]]).astype(
        np.int64)

    # transposed one-hot StT[j, b*128+s] = (dstloc[s, b] == j), bf16
    bf16 = _bf16()
    stT = np.zeros((NC_, P, totb * P), dtype=bf16)
    for c in range(NC_):
        dl_ = dstloc[c]                       # [s=128, b=totb]
        s_idx, b_idx = np.nonzero(dl_ >= 0.0)
        j_idx = dl_[s_idx, b_idx].astype(np.int64)
        stT[c][j_idx, b_idx * P + s_idx] = 1.0

    # both panel gathers are fully static: the inactive side gathers SH
    # safe junk rows (masked out later); no dynamic counts anywhere
    junk = np.arange(SH, dtype=np.int64)
    ipanL = np.zeros((NC_, P, SH // 16), dtype=np.int16)
    ipanH = np.zeros((NC_, P, SH // 16), dtype=np.int16)
    pmask = np.zeros((NC_, P, 2), dtype=np.float32)
    for c in range(NC_):
        rows = np.arange(c * SH, (c + 1) * SH, dtype=np.int64)
        if c * SH >= LSPL:
            ipanH[c] = _wrap16(rows - LSPL)
            ipanL[c] = _wrap16(junk)
            pmask[c, :, 1] = 1.0
        else:
            ipanL[c] = _wrap16(rows)
            ipanH[c] = _wrap16(junk)
            pmask[c, :, 0] = 1.0

    # batch (graph id) per local node slot; -1 on ghost slots
    batchloc = np.full((NC_, P, NW), -1.0, dtype=np.float32)
    for c in range(NC_):
        lo, hi = c * SH, min((c + 1) * SH, N)
        b = np.full(SH, -1.0, dtype=np.float32)
        if hi > lo:
            b[:hi - lo] = batch[lo:hi].astype(np.float32)
        batchloc[c] = b.reshape(NW, P).T

    return dict(
        blo=blo.astype(int), bhi=bhi.astype(int), bw=bw.astype(int),
        totb=totb, groups=groups, gbase=gbase, bwg=bwg, blog=blog,
        bwgmax=bwgmax, wblocks=wblocks,
        ntlo_g=ntlo_g, nthi_g=nthi_g, lo_off8=lo_off8, hi_off8=hi_off8,
        ilo=ilo, ihi=ihi, stT=stT, ipanL=ipanL, ipanH=ipanH, pmask=pmask,
        dstloc=dstloc, batchloc=batchloc,
    )


# c-major permutation: new col c*H+h holds original col h*C+c
_PERM = np.array([h * C + c for c in range(C) for h in range(H)], np.int64)


def _fold(Wm, a_s, a_d, b, perm_rows: bool):
    """[W(c-major cols) | A_src | A_dst] and matching extended bias."""
    K = Wm.shape[0]
    As = np.einsum("khc,hc->kh", Wm.reshape(K, H, C), a_s)
    Ad = np.einsum("khc,hc->kh", Wm.reshape(K, H, C), a_d)
    WR = np.concatenate([Wm[:, _PERM], As, Ad], axis=1).astype(np.float32)
    if perm_rows:
        WR = WR[_PERM]
    be = np.concatenate(
        [b[_PERM], np.einsum("hc,hc->h", b.reshape(H, C), a_s),
         np.einsum("hc,hc->h", b.reshape(H, C), a_d)]
    ).astype(np.float32)                                           # [264]
    return WR, be


def _build(meta, has_b1: bool):
    import concourse.bacc as bacc
    import concourse.mybir as mybir
    import concourse.tile as tile

    kq = int(os.environ.get("KQ", "1"))        # swdge queues
    kbg = int(os.environ.get("KBG", "2"))      # gather pool bufs

    f32 = mybir.dt.float32
    bf = mybir.dt.bfloat16
    i16 = mybir.dt.int16
    i32 = mybir.dt.int32
    Act = mybir.ActivationFunctionType
    Alu = mybir.AluOpType

    groups = meta["groups"]
    blog, bwg, gbase = meta["blog"], meta["bwg"], meta["gbase"]
    ntlo_g, nthi_g = meta["ntlo_g"], meta["nthi_g"]
    lo_off8, hi_off8 = meta["lo_off8"], meta["hi_off8"]
    wblocks = meta["wblocks"]
    TOTB, BWG = meta["totb"], meta["bwgmax"]
    MBW = int(meta["bw"].max())
    NLO8, NHI8 = int((ntlo_g // 16).sum()), int((nthi_g // 16).sum())

    nc = bacc.Bacc("TRN2", target_bir_lowering=False, debug=False,
                   num_devices=NC_, num_swdge_queues=kq)

    grp = [list(range(NC_))]

    def cc(kind, op, ins, outs):
        nc.gpsimd.collective_compute(
            kind, op, replica_groups=grp, ins=ins, outs=outs)

    # ------------------------------------------------------------- tensors
    xT = nc.dram_tensor("xT", [P, NP], bf, kind="ExternalInput")
    W1R = nc.dram_tensor("W1R", [IN, XC], bf, kind="ExternalInput")
    W2Ra = nc.dram_tensor("W2Ra", [P, XC], f32, kind="ExternalInput")
    W2Rb = nc.dram_tensor("W2Rb", [P, XC], f32, kind="ExternalInput")
    b2e = nc.dram_tensor("b2e", [P, XC], f32, kind="ExternalInput")
    Wlin = nc.dram_tensor("Wlin", [P, 4], f32, kind="ExternalInput")
    blin = nc.dram_tensor("blin", [G, 2], f32, kind="ExternalInput")
    iota128 = nc.dram_tensor("iota128", [P, P], bf, kind="ExternalInput")
    iota64 = nc.dram_tensor("iota64", [P, G], f32, kind="ExternalInput")
    ident = nc.dram_tensor("ident", [P, P], f32, kind="ExternalInput")
    identB = nc.dram_tensor("identB", [P, P], bf, kind="ExternalInput")
    dstloc = nc.dram_tensor("dstloc", [P, TOTB], bf, kind="ExternalInput")
    stT = nc.dram_tensor("stT", [P, TOTB * P], bf, kind="ExternalInput")
    idxlo = nc.dram_tensor("idxlo", [P, NLO8], i16, kind="ExternalInput")
    idxhi = nc.dram_tensor("idxhi", [P, NHI8], i16, kind="ExternalInput")
    idxpanL = nc.dram_tensor("idxpanL", [P, SH // 16], i16,
                             kind="ExternalInput")
    idxpanH = nc.dram_tensor("idxpanH", [P, SH // 16], i16,
                             kind="ExternalInput")
    pmask = nc.dram_tensor("pmask", [P, 2], f32, kind="ExternalInput")
    batchloc = nc.dram_tensor("batchloc", [P, NW], f32, kind="ExternalInput")
    if has_b1:
        b1e = nc.dram_tensor("b1e", [P, XC], f32, kind="ExternalInput")

    logits = nc.dram_tensor("logits", [G, 2], f32, kind="ExternalOutput")
    kdbg = os.environ.get("KDBG", "0") == "1"
    if kdbg:
        dbgel = nc.dram_tensor("dbgel", [SH, HC], f32, kind="ExternalOutput")
        dbgob = nc.dram_tensor("dbgob", [SH, HC], f32, kind="ExternalOutput")

    T1u = nc.dram_tensor("T1u", [NP, RW], bf, kind="Internal")
    T2s = nc.dram_tensor("T2s", [SH, RW], bf, kind="Internal")
    T2u = nc.dram_tensor("T2u", [NP, RW], bf, kind="Internal",
                         addr_space="Shared")
    prd = nc.dram_tensor("prd", [G, HC + 1], f32, kind="Internal")
    prs = nc.dram_tensor("prs", [G, HC + 1], f32, kind="Internal",
                         addr_space="Shared")

    with tile.TileContext(nc) as tc:
        with (
            tc.tile_pool(name="const", bufs=1) as cp,
            tc.tile_pool(name="work", bufs=3) as wp,
            tc.tile_pool(name="xw", bufs=2) as xp,
            tc.tile_pool(name="gat", bufs=kbg) as gp,
            tc.tile_pool(name="sel", bufs=2) as sp,
            tc.tile_pool(name="selt", bufs=2) as stp,
            tc.tile_pool(name="pan", bufs=1) as pnp,
            tc.tile_pool(name="adw", bufs=4) as awp,
            tc.tile_pool(name="msg", bufs=2) as mp,
            tc.tile_pool(name="outp", bufs=3) as op_,
            tc.tile_pool(name="ppre", bufs=2, space="PSUM") as ppre,
            tc.tile_pool(name="ptp", bufs=1, space="PSUM") as ptp,
            tc.tile_pool(name="pnum", bufs=2, space="PSUM") as pnum,
            tc.tile_pool(name="ppool", bufs=1, space="PSUM") as ppl,
            tc.tile_pool(name="pad", bufs=1, space="PSUM") as pap,
        ):
            # ---------------------------------------------------- constants
            def cload(dram, dt):
                tl = cp.tile(list(dram.shape), dt, tag=dram.name)
                nc.sync.dma_start(tl[:], dram[:])
                return tl

            w1r_t = cload(W1R, bf)
            w2a_t = cload(W2Ra, f32)
            w2b_t = cload(W2Rb, f32)
            b2e_t = cload(b2e, f32)
            wl_t = cload(Wlin, f32)
            bl_t = cload(blin, f32)
            io64_t = cload(iota64, f32)
            id_t = cload(ident, f32)
            idB_t = cload(identB, bf)
            ilo_t = cload(idxlo, i16)
            ihi_t = cload(idxhi, i16)
            ipanL_t = cload(idxpanL, i16)
            ipanH_t = cload(idxpanH, i16)
            pm_t = cload(pmask, f32)
            bat_t = cload(batchloc, f32)
            if has_b1:
                b1e_t = cload(b1e, f32)
            io1_t = cp.tile([P, 1, P], bf, tag="io1")
            nc.sync.dma_start(io1_t[:, 0, :], iota128[:])
            dst3_t = cp.tile([P, TOTB, 1], bf, tag="dst3")
            nc.sync.dma_start(
                dst3_t[:].rearrange("p b one -> p (b one)"), dstloc[:])

            # --------------------------------------------- layer-1 table
            # (full, built redundantly on every core, in global row order)
            def phase_a():
                nb = math.ceil(NP // P / 4)
                for bi in range(nb):
                    w0 = bi * 4
                    n = min(4, NP // P - w0)
                    g0 = w0 * P
                    xt = xp.tile([P, 4 * P], bf, tag="xt")
                    nc.sync.dma_start(xt[:, 0:n * P], xT[:, g0:g0 + n * P])
                    h4 = xp.tile([P, 4, EC], bf, tag="h4")
                    for k in range(n):
                        ps = ppre.tile([P, XC], f32, space="PSUM", tag="ppre")
                        nc.tensor.matmul(ps[:], lhsT=xt[:, k * P:(k + 1) * P],
                                         rhs=w1r_t[:], start=True, stop=True)
                        if has_b1:
                            nc.vector.tensor_tensor(
                                h4[:, k, 0:HC], ps[:, 0:HC],
                                b1e_t[:, 0:HC], op=Alu.add)
                            nc.vector.tensor_tensor(
                                h4[:, k, HC:EC].bitcast(f32),
                                ps[:, HC:XC], b1e_t[:, HC:XC], op=Alu.add)
                        else:
                            eng = (nc.scalar.copy, nc.vector.tensor_copy
                                   )[(bi * 4 + k) % 2]
                            eng(h4[:, k, 0:HC], ps[:, 0:HC])
                            nc.vector.tensor_copy(
                                h4[:, k, HC:EC].bitcast(f32), ps[:, HC:XC])
                    nc.sync.dma_start(
                        T1u[g0:g0 + n * P, 0:EC].rearrange(
                            "(k p) e -> p k e", p=P),
                        h4[:, 0:n, :])

            # ------------------------------------------------ panel gather
            # own shard's eval blocks -> a_dst[j] per window, [P, NW, H] bf16
            NWH = 25                      # panel windows per pass
            SHH = NWH * P                 # 3200 tokens per pass

            def panel1(Tu):
                adw = awp.tile([P, NW, H], bf, tag="adw")
                # each pass covers up to NWH windows; the inactive side
                # gathers 16 junk tokens per pass and is masked out
                for t0 in range(0, NW, NWH):
                    nwp = min(NWH, NW - t0)
                    ntp = nwp * P
                    ptL = pnp.tile([P, NWH, P], bf, tag="panL")
                    nc.gpsimd.dma_gather(
                        ptL[:, 0:nwp, :], Tu[0:LSPL, HC:HC + P],
                        ipanL_t[:, t0 * 8:(t0 + nwp) * 8],
                        ntp, ntp, P, elem_step=RW, single_packet=False)
                    ptH = pnp.tile([P, NWH, P], bf, tag="panH")
                    nc.gpsimd.dma_gather(
                        ptH[:, 0:nwp, :], Tu[LSPL:NP, HC:HC + P],
                        ipanH_t[:, t0 * 8:(t0 + nwp) * 8],
                        ntp, ntp, P, elem_step=RW, single_packet=False,
                        queue_num=1 % kq)
                    aw = awp.tile([P, NWH, H], f32, tag="aw32")
                    nc.vector.tensor_scalar(
                        aw[:, 0:nwp, :],
                        ptL[:, 0:nwp, :].bitcast(f32)[:, :, H:2 * H],
                        pm_t[:, 0:1], None, op0=Alu.mult)
                    ah = awp.tile([P, NWH, H], f32, tag="ah32")
                    nc.vector.tensor_scalar(
                        ah[:, 0:nwp, :],
                        ptH[:, 0:nwp, :].bitcast(f32)[:, :, H:2 * H],
                        pm_t[:, 1:2], None, op0=Alu.mult)
                    nc.vector.tensor_tensor(adw[:, t0:t0 + nwp, :],
                                            aw[:, 0:nwp, :],
                                            ah[:, 0:nwp, :], op=Alu.add)
                return adw

            def panel2():
                # T2s is core-local: the eval blocks are a plain DMA away
                adw = awp.tile([P, NW, H], bf, tag="adw")
                for t0 in range(0, NW, NWH):
                    nwp = min(NWH, NW - t0)
                    pt = pnp.tile([P, NWH, P], bf, tag="panL")
                    nc.sync.dma_start(
                        pt[:, 0:nwp, :],
                        T2s[t0 * P:(t0 + nwp) * P, HC:HC + P].rearrange(
                            "(w p) c -> p w c", p=P))
                    nc.vector.tensor_copy(
                        adw[:, t0:t0 + nwp, :],
                        pt[:, 0:nwp, :].bitcast(f32)[:, :, H:2 * H])
                return adw

            # shared window loop -----------------------------------------
            def window_loop(Tu, adw, sink):
                for gi, ws in enumerate(groups):
                    BL, BW = int(blog[gi]), int(bwg[gi])
                    b0 = int(gbase[gi])
                    NTL, NTH = int(ntlo_g[gi]), int(nthi_g[gi])
                    Gt = gp.tile([P, BWG, RW], bf, tag="G")
                    nc.gpsimd.dma_gather(
                        Gt[:, 0:BL, :], Tu[0:LSPL, :],
                        ilo_t[:, int(lo_off8[gi]):
                              int(lo_off8[gi]) + NTL // 16],
                        NTL, NTL, RW, single_packet=False)
                    nc.gpsimd.dma_gather(
                        Gt[:, BL:BW, :], Tu[LSPL:NP, :],
                        ihi_t[:, int(hi_off8[gi]):
                              int(hi_off8[gi]) + NTH // 16],
                        NTH, NTH, RW, single_packet=False,
                        queue_num=1 % kq)

                    # one-hot: S[p, b, j] = (j == dstloc[p, b]); and its
                    # transpose StT (host data) for a_dst routing
                    St = sp.tile([P, BWG, P], bf, tag="S")
                    nc.vector.tensor_tensor(
                        St[:, :BW, :],
                        io1_t[:].to_broadcast([P, BW, P]),
                        dst3_t[:, b0:b0 + BW, :].to_broadcast([P, BW, P]),
                        op=Alu.is_equal)
                    StT = stp.tile([P, BWG, P], bf, tag="ST")
                    nc.sync.dma_start(
                        StT[:, 0:BW, :].rearrange("p b s -> p (b s)"),
                        stT[:, b0 * P:(b0 + BW) * P])

                    # a_dst per edge slot: adp[s, b, h] = sum_j StT[j, b, s]
                    # * adw[j, w(b), h]  (tiny per-block PE matmuls)
                    adp = pap.tile([P, BWG, H], f32, space="PSUM", tag="ad")
                    for wi in ws:
                        for r0, r1 in wblocks[wi]:
                            for b in range(r0, r1):
                                nc.tensor.matmul(
                                    adp[:, b - b0, :],
                                    lhsT=StT[:, b - b0, :],
                                    rhs=adw[:, wi, :],
                                    start=True, stop=True)

                    # e = a_src(gathered) + a_dst(routed);
                    # exp(leaky(e)) as exp(0.6*(e + (2/3)|e|))
                    ev = mp.tile([P, BWG, H], f32, tag="ev")
                    GtF = Gt[:].bitcast(f32)
                    nc.vector.tensor_tensor(ev[:, :BW, :],
                                            GtF[:, :BW, P:P + H],
                                            adp[:, :BW, :], op=Alu.add)
                    av = mp.tile([P, BWG, H], f32, tag="av")
                    nc.scalar.activation(av[:, :BW, :], ev[:, :BW, :],
                                         Act.Abs, scale=2.0 / 3.0)
                    nc.vector.tensor_tensor(av[:, :BW, :], ev[:, :BW, :],
                                            av[:, :BW, :], op=Alu.add)
                    nc.vector.tensor_scalar(av[:, :BW, :], av[:, :BW, :],
                                            60.0, None, op0=Alu.min)
                    ex = mp.tile([P, BWG, H], bf, tag="ex")
                    nc.scalar.activation(ex[:, :BW, :], av[:, :BW, :],
                                         Act.Exp, scale=0.6)

                    for wi in ws:
                        # msg = [h * ex | ex]; c-major h -> packed 4-wide
                        # last dim for the DVE fast mode
                        (lor, hir) = wblocks[wi]
                        blocks = list(range(*lor)) + list(range(*hir))
                        nb_w = len(blocks)
                        msg = mp.tile([P, MBW, HC + H], bf, tag="msg")
                        for j0, (r0, r1) in ((0, lor),
                                             (lor[1] - lor[0], hir)):
                            if r1 == r0:
                                continue
                            nbr = r1 - r0
                            nc.vector.tensor_tensor(
                                msg[:, j0:j0 + nbr, 0:HC].rearrange(
                                    "p b (c h) -> p b c h", c=C),
                                Gt[:, r0 - b0:r1 - b0, 0:HC].rearrange(
                                    "p b (c h) -> p b c h", c=C),
                                ex[:, r0 - b0:r1 - b0, :].rearrange(
                                    "p b h -> p b () h").to_broadcast(
                                        [P, nbr, C, H]),
                                op=Alu.mult)
                            nc.scalar.copy(
                                msg[:, j0:j0 + nbr, HC:HC + H],
                                ex[:, r0 - b0:r1 - b0, :])
                        nmp = pnum.tile([P, HC + H], f32, space="PSUM",
                                        tag="nm")
                        for i, b in enumerate(blocks):
                            nc.tensor.matmul(nmp[:], lhsT=St[:, b - b0, :],
                                             rhs=msg[:, i, 0:HC + H],
                                             start=(i == 0),
                                             stop=(i == nb_w - 1))
                        rd = mp.tile([P, H], f32, tag="rd")
                        nc.vector.tensor_scalar(rd[:], nmp[:, HC:HC + H],
                                                1e-30, None, op0=Alu.max)
                        nc.vector.reciprocal(rd[:], rd[:])
                        ob = op_.tile([P, HC], f32, tag="ob")
                        nc.vector.tensor_tensor(
                            ob[:].rearrange("p (c h) -> p c h", c=C),
                            nmp[:, 0:HC].rearrange("p (c h) -> p c h", c=C),
                            rd[:].rearrange("p h -> p () h").to_broadcast(
                                [P, C, H]),
                            op=Alu.mult)
                        sink(wi, ob)

            # ---------------- layer-1 sink: fused layer-2 table build.
            # elu(x) = relu(x) + (exp(-relu(-x)) - 1), Act-heavy form.
            def elu1(ob, pool):
                t0 = pool.tile([P, HC], f32, tag="elu0")
                nc.scalar.activation(t0[:], ob[:], Act.Relu, scale=-1.0)
                nc.scalar.activation(t0[:], t0[:], Act.Exp, scale=-1.0)
                t1 = pool.tile([P, HC], f32, tag="elu1")
                nc.scalar.activation(t1[:], ob[:], Act.Relu)
                return t0, t1

            def sink1(w, ob):
                t0, t1 = elu1(ob, wp)
                nc.vector.tensor_scalar(t0[:], t0[:], -1.0, None,
                                        op0=Alu.add)
                el = wp.tile([P, HC], f32, tag="el1")
                nc.vector.tensor_tensor(el[:], t1[:], t0[:], op=Alu.add)
                ps2 = ppre.tile([P, XC], f32, space="PSUM", tag="ppre")
                for k in range(2):
                    tp = ptp.tile([P, P], f32, space="PSUM", tag="tp")
                    nc.tensor.transpose(tp[:], el[:, k * P:(k + 1) * P],
                                        id_t[:])
                    et = wp.tile([P, P], f32, tag="eT")
                    nc.scalar.copy(et[:], tp[:])
                    nc.tensor.matmul(ps2[:], lhsT=et[:],
                                     rhs=(w2a_t if k == 0 else w2b_t)[:],
                                     start=(k == 0), stop=(k == 1))
                h2 = wp.tile([P, EC], bf, tag="h2")
                nc.vector.tensor_tensor(h2[:, 0:HC], ps2[:, 0:HC],
                                        b2e_t[:, 0:HC], op=Alu.add)
                nc.vector.tensor_tensor(h2[:, HC:EC].bitcast(f32),
                                        ps2[:, HC:XC],
                                        b2e_t[:, HC:XC], op=Alu.add)
                nc.sync.dma_start(T2s[w * P:(w + 1) * P, 0:EC], h2[:])
                if w == NW - 1:
                    cc("AllGather", Alu.bypass,
                       ins=[T2s[:, :]], outs=[T2u[:, :]])

            # -------------------------------- layer-2 sink: mean pooling
            plp = ppl.tile([G, HC + 1], f32, space="PSUM", tag="pool")

            def sink2(w, ob):
                t0, t1 = elu1(ob, op_)
                nc.vector.tensor_scalar(t0[:], t0[:], -1.0, None,
                                        op0=Alu.add)
                el = op_.tile([P, HC + 1], f32, tag="el2")
                nc.vector.tensor_tensor(el[:, 0:HC], t1[:], t0[:],
                                        op=Alu.add)
                nc.vector.memset(el[:, HC:HC + 1], 1.0)
                bm = op_.tile([P, G], f32, tag="bm")
                nc.vector.tensor_scalar(bm[:], io64_t[:],
                                        bat_t[:, w:w + 1], None,
                                        op0=Alu.is_equal)
                nc.tensor.matmul(plp[:], lhsT=bm[:], rhs=el[:],
                                 start=(w == 0), stop=(w == NW - 1))
                if kdbg:
                    nc.sync.dma_start(dbgel[w * P:(w + 1) * P, :],
                                      el[:, 0:HC])
                    nc.sync.dma_start(dbgob[w * P:(w + 1) * P, :], ob[:])

            def epilogue():
                pls = wp.tile([G, HC + 1], f32, tag="pls")
                nc.vector.tensor_copy(pls[:], plp[:])
                nc.sync.dma_start(prd[:, :], pls[:])
                cc("AllReduce", Alu.add, ins=[prd[:, :]], outs=[prs[:, :]])
                pr = wp.tile([G, HC + 1], f32, tag="pr")
                nc.sync.dma_start(pr[:], prs[:, :])
                cnt = wp.tile([G, 1], f32, tag="cnt")
                nc.vector.tensor_scalar(cnt[:], pr[:, HC:HC + 1], 1.0,
                                        None, op0=Alu.max)
                nc.vector.reciprocal(cnt[:], cnt[:])
                pooled = wp.tile([G, HC], f32, tag="pooled")
                nc.vector.tensor_scalar(pooled[:], pr[:, 0:HC],
                                        cnt[:, 0:1], None, op0=Alu.mult)
                psl_full = ppre.tile([P, XC], f32, space="PSUM", tag="ppre")
                psl = psl_full[0:G, 0:2]
                for k in range(2):
                    tp = ptp.tile([P, P], f32, space="PSUM", tag="tpf")
                    nc.tensor.transpose(tp[:, 0:G],
                                        pooled[:, k * P:(k + 1) * P],
                                        id_t[0:G, 0:G])
                    pt = wp.tile([P, G], f32, tag="pT")
                    nc.scalar.copy(pt[:], tp[:, 0:G])
                    nc.tensor.matmul(psl, lhsT=pt[:],
                                     rhs=wl_t[:, 2 * k:2 * k + 2],
                                     start=(k == 0), stop=(k == 1))
                lg = wp.tile([G, 2], f32, tag="lg")
                nc.vector.tensor_tensor(lg[:], psl, bl_t[:], op=Alu.add)
                nc.sync.dma_start(logits[:, :], lg[:])

            phase_a()
            adw1 = panel1(T1u)
            window_loop(T1u, adw1, sink1)
            adw2 = panel1(T2u) if os.environ.get('KP2','0')=='1' else panel2()
            window_loop(T2u, adw2, sink2)
            epilogue()

    nc.compile()
    return nc


def kernel(**inputs):
    from concourse.bass_utils import run_bass_kernel_spmd

    nc, in_maps = prepare(inputs)
    res = run_bass_kernel_spmd(nc, in_maps, core_ids=list(range(NC_)))
    return res.results[0]["logits"]


def prepare(inputs):
    bf16 = _bf16()
    x = np.asarray(inputs["x"], np.float32)
    edge_index = np.asarray(inputs["edge_index"], np.int64)
    batch = np.asarray(inputs["batch"], np.int64)
    W1 = np.asarray(inputs["W1"], np.float32)
    W2 = np.asarray(inputs["W2"], np.float32)
    W_lin = np.asarray(inputs["W_lin"], np.float32)
    b1 = np.asarray(inputs["b1"], np.float32)
    b2 = np.asarray(inputs["b2"], np.float32)
    b_lin = np.asarray(inputs["b_lin"], np.float32)
    a_src1 = np.asarray(inputs["a_src1"], np.float32)
    a_dst1 = np.asarray(inputs["a_dst1"], np.float32)
    a_src2 = np.asarray(inputs["a_src2"], np.float32)
    a_dst2 = np.asarray(inputs["a_dst2"], np.float32)

    has_b1 = bool(np.any(b1))
    meta = _preprocess(edge_index, batch)
    nc = _build(meta, has_b1)

    W1R, b1ext = _fold(W1, a_src1, a_dst1, b1, perm_rows=False)
    W2R, b2ext = _fold(W2, a_src2, a_dst2, b2, perm_rows=True)
    b2eff = b2ext
    wlin_p = W_lin[_PERM]
    blin_eff = b_lin

    iota128 = np.tile(np.arange(P, dtype=np.float32), (P, 1))
    iota64 = np.tile(np.arange(G, dtype=np.float32), (P, 1))
    ident = np.eye(P, dtype=np.float32)
    identB = np.eye(P, dtype=np.float32).astype(bf16)
    wlin_2 = np.concatenate([wlin_p[0:P], wlin_p[P:2 * P]], axis=1)

    # full padded x, transposed: [IN, NP]; same for every core
    xs = np.zeros((P, NP), np.float32)
    xs[:, 0:N] = x.T
    xs = xs.astype(bf16)

    in_maps = []
    for c in range(NC_):
        im = {
            "xT": xs,
            "W1R": W1R.astype(bf16),
            "W2Ra": np.ascontiguousarray(W2R[0:P]),
            "W2Rb": np.ascontiguousarray(W2R[P:2 * P]),
            "b2e": np.tile(b2eff, (P, 1)),
            "Wlin": np.ascontiguousarray(wlin_2),
            "blin": np.tile(blin_eff, (G, 1)),
            "iota128": iota128.astype(bf16),
            "iota64": iota64,
            "ident": ident, "identB": identB,
            "dstloc": np.ascontiguousarray(meta["dstloc"][c]).astype(bf16),
            "stT": np.ascontiguousarray(meta["stT"][c]),
            "idxlo": np.ascontiguousarray(meta["ilo"][c]),
            "idxhi": np.ascontiguousarray(meta["ihi"][c]),
            "idxpanL": np.ascontiguousarray(meta["ipanL"][c]),
            "idxpanH": np.ascontiguousarray(meta["ipanH"][c]),
            "pmask": np.ascontiguousarray(meta["pmask"][c]),
            "batchloc": np.ascontiguousarray(meta["batchloc"][c]),
        }
        if has_b1:
            im["b1e"] = np.tile(b1ext, (P, 1))
        in_maps.append(im)

    return nc, in_maps
